# revision 1
# baseline (speedup 1.0000x reference)
"""CRF NLL loss kernel for Trainium2 (8 NeuronCores, batch-parallel).

Strategy: shard the 4096-sentence batch across 8 cores (512 each). Per core,
run the CRF forward recursion in probability space with tags on partitions:
126 partitions = 14 groups x 9 body-tags; block-diagonal exp(transitions) as
stationary PE weights; each time step is one matmul (PE) + one elementwise
multiply by exp(feats) (DVE). The gold path score is computed by a parallel
"beta" recursion (one-hot-masked emission factors selects exactly the gold
path term of the forward sum), so forward and gold share the same per-step
instructions on a 74-wide free axis (37 alpha sentences | 37 beta sentences
per group). A log-scale carry C is maintained by rescaling every 32 steps
(the ScalarE Ln LUT loses precision for large-magnitude inputs, so sums are
kept in a narrow range and pre-scaled by 2^-16 before Ln; the ln-offsets
cancel between the alpha and beta halves).
"""
import os
import sys

import numpy as np

sys.path.insert(0, "/opt/trn_rl_repo")

from contextlib import ExitStack

import concourse.bacc as bacc
import concourse.bass as bass
import concourse.tile as tile
from concourse import mybir
from concourse.bass_utils import run_bass_kernel_spmd

# problem constants (hardcoded per spec)
B, T, K = 4096, 2048, 11
START, STOP = 10, 9
NCORES = 8
BL = B // NCORES          # 512 sentences per core
G, KT, J = 14, 9, 37      # groups x body-tags x sentences-per-group (518 slots)
P = 128                   # padded partitions (126 live = G*KT, 2 dead)
PL = G * KT               # live partitions
W = 2 * J                 # 74 free: [alpha | beta]
TC = 128                  # chunk length
NCHUNK = T // TC
RS = 32                   # rescale cadence (steps)
LNSCALE = 2.0 ** -18      # pre-scale for ACT Ln (valid range is +-2^64);
                          # the ln(2^-32) offsets cancel between halves
C0A, C0B = 3.2, 0.5       # per-step log recentering for alpha / beta chains

F32 = mybir.dt.float32
BF16 = mybir.dt.bfloat16
I8 = mybir.dt.int8


def _build_nc(nrep=1):
    nc = bacc.Bacc()
    f_in = nc.declare_dram_parameter("feats_t", [P, T, J], F32, isOutput=False)
    g_in = nc.declare_dram_parameter("tags_t", [P, T, J], I8, isOutput=False)
    bd_in = nc.declare_dram_parameter("bd_lhst", [P, P], BF16, isOutput=False)
    astart_in = nc.declare_dram_parameter("astart", [P, 1], F32, isOutput=False)
    astop_in = nc.declare_dram_parameter("astop", [P, G], BF16, isOutput=False)
    ones_in = nc.declare_dram_parameter("ones_bd", [P, G], BF16, isOutput=False)
    bcast_in = nc.declare_dram_parameter("bcast", [G, P], F32, isOutput=False)
    kcol_in = nc.declare_dram_parameter("kcol", [P, 1], I8, isOutput=False)
    out_ext = nc.declare_dram_parameter("nll", [G, J], F32, isOutput=True)

    with tile.TileContext(nc) as tc, ExitStack() as ctx:
        consts = ctx.enter_context(tc.tile_pool(name="consts", bufs=1))
        feats_pool = ctx.enter_context(tc.tile_pool(name="feats", bufs=2))
        tags_pool = ctx.enter_context(tc.tile_pool(name="tags", bufs=2))
        e_pool = ctx.enter_context(tc.tile_pool(name="ecomb", bufs=2))
        state_pool = ctx.enter_context(tc.tile_pool(name="state", bufs=3))
        small_pool = ctx.enter_context(tc.tile_pool(name="small", bufs=2))
        psum_pool = ctx.enter_context(
            tc.tile_pool(name="psum", bufs=4, space="PSUM"))

        bd = consts.tile([P, P], BF16)
        nc.sync.dma_start(out=bd, in_=bd_in[:])
        astart = consts.tile([P, 1], F32)
        nc.sync.dma_start(out=astart, in_=astart_in[:])
        astop = consts.tile([P, G], BF16)
        nc.sync.dma_start(out=astop, in_=astop_in[:])
        ones_bd = consts.tile([P, G], BF16)
        nc.sync.dma_start(out=ones_bd, in_=ones_in[:])
        bcast = consts.tile([G, P], F32)
        nc.sync.dma_start(out=bcast, in_=bcast_in[:])
        kcol = consts.tile([P, 1], I8)
        nc.sync.dma_start(out=kcol, in_=kcol_in[:])

        cacc = consts.tile([G, W], F32)
        nc.vector.memset(cacc, 0.0)
        bias_a = consts.tile([P, 1], F32)
        nc.vector.memset(bias_a, -C0A)
        bias_b = consts.tile([P, 1], F32)
        nc.vector.memset(bias_b, -C0B)
        lnscale = consts.tile([G, 1], F32)
        nc.vector.memset(lnscale, LNSCALE)

        alpha = None
        for rep in range(nrep):
          for chunk in range(NCHUNK):
            ft = feats_pool.tile([P, TC, J], F32, tag="ft")
            nc.sync.dma_start(out=ft, in_=f_in[:, chunk * TC:(chunk + 1) * TC, :])
            tg = tags_pool.tile([P, TC, J], I8, tag="tg")
            nc.sync.dma_start(out=tg, in_=g_in[:, chunk * TC:(chunk + 1) * TC, :])
            ec = e_pool.tile([P, TC, W], F32, tag="ec")
            nc.scalar.activation(
                out=ec[:, :, 0:J], in_=ft,
                func=mybir.ActivationFunctionType.Exp, bias=bias_a, scale=1.0)
            nc.scalar.activation(
                out=ec[:, :, J:W], in_=ft,
                func=mybir.ActivationFunctionType.Exp, bias=bias_b, scale=1.0)
            # beta half: keep only the gold-tag emission factor
            nc.vector.scalar_tensor_tensor(
                out=ec[:, :, J:W], in0=tg, scalar=kcol, in1=ec[:, :, J:W],
                op0=mybir.AluOpType.is_equal, op1=mybir.AluOpType.mult)

            for t in range(TC):
                tau = chunk * TC + t
                if tau == 0:
                    alpha = state_pool.tile([P, W], BF16, tag="alpha")
                    nc.vector.tensor_scalar_mul(
                        out=alpha, in0=ec[:, 0, :], scalar1=astart)
                else:
                    ps = psum_pool.tile([P, W], F32, tag="ps")
                    nc.tensor.matmul(ps, bd, alpha, start=True, stop=True)
                    alpha = state_pool.tile([P, W], BF16, tag="alpha")
                    nc.vector.tensor_mul(out=alpha, in0=ps, in1=ec[:, t, :])

                # Rescale off the critical chain: measure S = sum_k alpha at
                # local steps {24,56,88,120}, then fold 1/S into the emission
                # slice 4 steps ahead (ec[:, t+4, :]) — the serial PE<->DVE
                # chain is never blocked, and the DVE scale-mul rides the DVE
                # program order (no extra cross-engine hops).
                if t % 32 == 24:
                    s_ps = psum_pool.tile([G, W], F32, tag="sps")
                    nc.tensor.matmul(s_ps, ones_bd, alpha, start=True, stop=True)
                    r_sb = small_pool.tile([G, W], F32, tag="r")
                    nc.vector.reciprocal(out=r_sb, in_=s_ps)
                    ln_sb = small_pool.tile([G, W], F32, tag="ln")
                    nc.scalar.activation(
                        out=ln_sb, in_=s_ps,
                        func=mybir.ActivationFunctionType.Ln, scale=lnscale)
                    nc.vector.tensor_add(out=cacc, in0=cacc, in1=ln_sb)
                    rb_ps = psum_pool.tile([P, W], F32, tag="ps")
                    nc.tensor.matmul(rb_ps, bcast, r_sb, start=True, stop=True)
                    rb_sb = state_pool.tile([P, W], BF16, tag="rb")
                    nc.scalar.activation(
                        out=rb_sb, in_=rb_ps,
                        func=mybir.ActivationFunctionType.Copy)
                    nc.vector.tensor_mul(
                        out=ec[:, t + 4, :], in0=ec[:, t + 4, :], in1=rb_sb)

        f_ps = psum_pool.tile([G, W], F32, tag="sps")
        nc.tensor.matmul(f_ps, astop, alpha, start=True, stop=True)
        ln_f = small_pool.tile([G, W], F32, tag="ln")
        nc.scalar.activation(
            out=ln_f, in_=f_ps, func=mybir.ActivationFunctionType.Ln,
            scale=lnscale)
        nc.vector.tensor_add(out=cacc, in0=cacc, in1=ln_f)

        nll_sb = small_pool.tile([G, J], F32, tag="nll")
        nc.vector.tensor_sub(out=nll_sb, in0=cacc[:, 0:J], in1=cacc[:, J:W])
        nc.vector.tensor_scalar_add(
            out=nll_sb, in0=nll_sb, scalar1=float(T) * (C0A - C0B))
        nc.sync.dma_start(out=out_ext[:], in_=nll_sb)

    nc.finalize()
    return nc


def _host_prep(feats, tags, transitions):
    """Build per-core input maps. Layout/dtype staging only — all FLOPs on device
    except the 11x11 exp(transitions) weight build."""
    import ml_dtypes
    f32 = np.float32
    bf16 = ml_dtypes.bfloat16
    feats = np.asarray(feats, dtype=f32)
    tags_i = np.asarray(tags).astype(np.int8)
    trans = np.asarray(transitions, dtype=f32)

    def padp(a):
        """pad partition (first) dim from PL=126 to P=128 with zeros"""
        out = np.zeros((P,) + a.shape[1:], dtype=a.dtype)
        out[:PL] = a
        return np.ascontiguousarray(out)

    A = np.exp(trans.astype(np.float64)).astype(f32)     # A[next, prev]
    Abody = A[:KT, :KT]
    eye = np.eye(G, dtype=f32)
    bd0 = np.kron(eye, Abody.T)                          # [126,126]
    bd = np.zeros((P, P), dtype=bf16)
    bd[:PL, :PL] = bd0.astype(bf16)
    astart = padp(np.tile(A[:KT, START], G)[:, None].astype(f32))
    astop = padp(np.kron(eye, A[STOP, :KT].reshape(KT, 1)).astype(bf16))
    ones_bd = padp(np.kron(eye, np.ones((KT, 1), f32)).astype(bf16))
    bcast = np.zeros((G, P), dtype=f32)
    bcast[:, :PL] = np.kron(eye, np.ones((1, KT), f32))
    kcol = padp(np.tile(np.arange(KT, dtype=np.int8), G)[:, None])
    kcol[PL:] = -1

    nslots = G * J
    in_maps = []
    for c in range(NCORES):
        fb = feats[c * BL:(c + 1) * BL, :, :KT]
        tb = tags_i[c * BL:(c + 1) * BL]
        fpad = np.zeros((nslots, T, KT), dtype=f32)
        fpad[:BL] = fb
        tpad = np.zeros((nslots, T), dtype=np.int8)
        tpad[:BL] = tb
        feats_T = padp(
            fpad.reshape(G, J, T, KT).transpose(0, 3, 2, 1).reshape(PL, T, J))
        tags_T = padp(
            np.ascontiguousarray(np.broadcast_to(
                tpad.reshape(G, J, T).transpose(0, 2, 1)[:, None, :, :],
                (G, KT, T, J))).reshape(PL, T, J))
        in_maps.append({
            "feats_t": feats_T,
            "tags_t": tags_T,
            "bd_lhst": bd,
            "astart": astart,
            "astop": astop,
            "ones_bd": ones_bd,
            "bcast": bcast,
            "kcol": kcol,
        })
    return in_maps


LAST_EXEC_NS = None


def kernel(feats, tags, transitions):
    global LAST_EXEC_NS
    in_maps = _host_prep(feats, tags, transitions)
    nc = _build_nc()
    trace = os.environ.get("KERNEL_TRACE") == "1"
    res = None
    for attempt in range(3):
        try:
            res = run_bass_kernel_spmd(
                nc, in_maps, list(range(NCORES)), trace=trace)
            break
        except Exception:
            if attempt == 2:
                raise
            # the device occasionally reports NRT_EXEC_UNIT_UNRECOVERABLE;
            # resetting the PJRT client (like a fresh process) recovers it
            import time as _time
            import jax as _jax
            try:
                _jax.clear_caches()
            except Exception:
                pass
            for fn in ("clear_backends",):
                try:
                    getattr(_jax.extend.backend, fn)()
                except Exception:
                    try:
                        getattr(_jax, fn)()
                    except Exception:
                        pass
            _time.sleep(5)
    LAST_EXEC_NS = res.exec_time_ns
    outs = []
    for c in range(NCORES):
        nll_parts = np.asarray(res.results[c]["nll"], dtype=np.float32)
        outs.append(nll_parts.reshape(-1)[:BL])
    return np.concatenate(outs).astype(np.float32)


if __name__ == "__main__":
    rng = np.random.default_rng(0)
    feats = rng.standard_normal((B, T, K), dtype=np.float32)
    tags = rng.integers(0, 9, size=(B, T), dtype=np.int64)
    trans = rng.random((K, K), dtype=np.float32)
    trans[START, :] = -10000.0
    trans[:, STOP] = -10000.0
    out = kernel(feats=feats, tags=tags, transitions=trans)
    print(out.shape, out[:4])



# revision 28
# speedup vs baseline: 3.8668x; 3.8668x over previous
"""CRF NLL loss kernel for Trainium2 (8 NeuronCores, batch-parallel).

Strategy (v3): time-segmented forward recursion. The per-step chain
matmul(PE) -> emission-multiply has ~430ns serial latency, so a single
T=2048 chain is latency-bound (~1ms). The CRF transfer operator is strongly
contracting (Birkhoff coefficient <= tanh(0.5) ~ 0.46/step since
log A in [0,1)), so the forward direction forgets its init in ~8 steps. We
split T into S=16 segments of 128 steps; each segment chain starts BI=8
steps early (alpha: uniform init; gold/beta: exact one-hot at the known
gold tag) and per-segment results are stitched by a telescoping boundary
correction validated in proto.py:

  logX = sum_s ln( fin_sum_s * prod(rescale sums after start) / sum@start )

Chains fuse F segments per instruction and run staggered. GPSIMD cannot
touch PSUM on real HW, so the PSUM crossing (matmul output -> SBUF) runs on
DVE ("v" chains: fused multiply) or ACT ("a" chains: copy, then Pool does
the SBUF multiply). The gold half's masked emission ec_beta = onehot *
ec_alpha is built by Pool from host-packed one-hot BITS:
(packed & (1<<q)) * ec_alpha — the stray 2^q factor is a compile-time
constant absorbed into the final nll offset. Scale accumulators are kept in
product form (one Ln per chain at the end) to avoid ACT function-table
swaps; per-half centering immediates keep everything in f32/bf16 range.
"""
import math
import os
import sys

import numpy as np

sys.path.insert(0, "/opt/trn_rl_repo")

from contextlib import ExitStack

import concourse.bacc as bacc
import concourse.bass as bass
import concourse.tile as tile
from concourse import mybir
from concourse.bass_utils import run_bass_kernel_spmd

# problem constants (hardcoded per spec)
B, T, K = 4096, 2048, 11
START, STOP = 10, 9
NCORES = 8
BL = B // NCORES          # 512 sentences per core
G, KT, J = 14, 9, 37      # groups x body-tags x sentences-per-group (518 slots)
P = 128
PL = G * KT               # live partitions
W = 2 * J                 # 74 free per segment: [alpha | beta]

# segmentation
S = 16                    # segments
SEG = T // S              # 128 official steps per segment
BI = 8                    # burn-in steps
N = SEG + BI              # chain steps
LB = 8                    # steps per ec/ft block
NB = N // LB              # blocks per chain
RS = 16                   # rescale cadence (beta ~e^-2.7/step)
RPH = 11                  # rescale at n in {11,27,...,123}
NBUND = 8                 # bundles per chain
TP = 17 * SEG             # padded time length (BI junk + T + tail junk)
C0A = 3.2                 # per-step log recentering (shared by both halves)

# chain layout: list of (cross_engine, segments). "v": DVE fused multiply;
# "a": ACT copy + Pool multiply. Each chain's segments must be equally
# spaced (the block DMA uses one strided access pattern).
SEGMAP = [[0, 2, 4, 6, 8, 10, 12, 14], [1, 3, 5, 7, 9, 11, 13, 15]]
KINDS = ["v", "v"]
C = len(SEGMAP)
CHAINS = [(KINDS[c], len(SEGMAP[c])) for c in range(C)]
assert sorted(s for segs in SEGMAP for s in segs) == list(range(S))
for segs in SEGMAP:
    assert len(set(np.diff(segs))) <= 1

# Ln LUT inputs are pre-scaled to land near 1 (the LUT loses precision at
# extreme magnitudes). The alpha half stays near 1 (scale 1.0); the beta
# (gold one-hot) half decays ~e^-2.7/step: its sums sit near e^-21 at the
# snapshot (~8 steps), e^-32 at the first bundle, e^-42 at later bundles
# (16-step windows) and e^-32 at the final read (12 steps). The ln(scale)
# offsets are per-column-class constants folded into NLL_CONST.
SBI_B, SBU1_B, SBU_B, SFI_B = 21.0, 32.0, 42.0, 32.0
BETA_LN_SUM = SBU1_B + (NBUND - 1) * SBU_B + SFI_B
# generic beta segment: res_b = true + (-SBI_B + BETA_LN_SUM)
# segment 0 beta: no snapshot term: res_b = true + BETA_LN_SUM
NLL_CONST = (S - 1) * (-SBI_B + BETA_LN_SUM) + BETA_LN_SUM

F32 = mybir.dt.float32
BF16 = mybir.dt.bfloat16
U8 = mybir.dt.uint8


def _seg_list(c):
    return SEGMAP[c]


def _build_nc(nrep=1):
    nc = bacc.Bacc()
    f_in = nc.declare_dram_parameter("feats_t", [P, TP, J], BF16, isOutput=False)
    oh_ins = [nc.declare_dram_parameter(f"onehot{c}",
                                        [P, NB, CHAINS[c][1], LB, J],
                                        U8, isOutput=False) for c in range(C)]
    bd_in = nc.declare_dram_parameter("bd_lhst", [P, P], BF16, isOutput=False)
    astart_in = nc.declare_dram_parameter("astart", [P, 1], F32, isOutput=False)
    astop_in = nc.declare_dram_parameter("astop", [P, G], BF16, isOutput=False)
    ones_in = nc.declare_dram_parameter("ones_bd", [P, G], BF16, isOutput=False)
    bcast_in = nc.declare_dram_parameter("bcast", [G, P], F32, isOutput=False)
    init_ins = [nc.declare_dram_parameter(f"init_st{c}", [P, CHAINS[c][1], W],
                                          BF16, isOutput=False)
                for c in range(C)]
    out_ext = nc.declare_dram_parameter("nll", [G, J], F32, isOutput=True)
    dbg_steps = [0, 1, 2, 7, 8, 9, 19, 20] if os.environ.get("KDBG") else []
    dbg_outs = {n: nc.declare_dram_parameter(f"dbg{n}", [P, CHAINS[1][1], W],
                                             F32, isOutput=True)
                for n in dbg_steps}
    dbg_ec = {n: nc.declare_dram_parameter(f"dbgec{n}", [P, CHAINS[1][1], W],
                                           F32, isOutput=True)
              for n in dbg_steps}
    dbg_acc = [nc.declare_dram_parameter(f"dbgacc{c}", [G, CHAINS[c][1], W],
                                         F32, isOutput=True)
               for c in range(C)] if dbg_steps else []
    dbg_res = [nc.declare_dram_parameter(f"dbgres{c}", [G, CHAINS[c][1], W],
                                         F32, isOutput=True)
               for c in range(C)] if dbg_steps else []

    f_r = f_in.rearrange("p (w q) j -> p w q j", q=SEG)   # [P, 17, 128, J]

    with tile.TileContext(nc) as tc, ExitStack() as ctx:
        consts = ctx.enter_context(tc.tile_pool(name="consts", bufs=1))
        ft_pools = [ctx.enter_context(tc.tile_pool(name=f"ft{c}", bufs=2))
                    for c in range(C)]
        ec_pools = [ctx.enter_context(tc.tile_pool(name=f"ec{c}", bufs=2))
                    for c in range(C)]
        state_pools = [ctx.enter_context(tc.tile_pool(name=f"st{c}", bufs=3))
                       for c in range(C)]
        sbx_pools = [ctx.enter_context(tc.tile_pool(name=f"sbx{c}", bufs=2))
                     for c in range(C)]
        small_pool = ctx.enter_context(tc.tile_pool(name="small", bufs=4))
        cps_pools = [ctx.enter_context(
            tc.tile_pool(name=f"cpsum{c}", bufs=1, space="PSUM"))
            for c in range(C)]
        bps_pool = ctx.enter_context(
            tc.tile_pool(name="bpsum", bufs=2, space="PSUM"))

        # ---- constants ----
        bd = consts.tile([P, P], BF16, name="bd")
        nc.sync.dma_start(out=bd, in_=bd_in[:])
        astart = consts.tile([P, 1], F32, name="astart")
        nc.sync.dma_start(out=astart, in_=astart_in[:])
        astop = consts.tile([P, G], BF16, name="astop")
        nc.sync.dma_start(out=astop, in_=astop_in[:])
        ones_bd = consts.tile([P, G], BF16, name="ones_bd")
        nc.sync.dma_start(out=ones_bd, in_=ones_in[:])
        bcast = consts.tile([G, P], F32, name="bcast")
        nc.sync.dma_start(out=bcast, in_=bcast_in[:])
        bias_a = consts.tile([P, 1], F32, name="bias_a")
        nc.vector.memset(bias_a, -C0A)
        oh_pools = [ctx.enter_context(tc.tile_pool(name=f"oh{c}", bufs=2))
                    for c in range(C)]
        OHQ = 5                           # blocks per one-hot staging DMA
        oh_tiles = [None] * C

        def stage_onehot(c, b):
            Fc = CHAINS[c][1]
            nblk = min(OHQ, NB - b)
            oh = oh_pools[c].tile([P, OHQ, Fc, LB, J], U8, name="oh",
                                  tag="oh")
            nc.sync.dma_start(out=oh[:, 0:nblk], in_=oh_ins[c][:, b:b + nblk])
            return oh

        accs, ress = [], []
        for c in range(C):
            Fc = CHAINS[c][1]
            acc = consts.tile([G, Fc, W], F32, name="acc", tag=f"acc{c}")
            accs.append(acc)
            res = consts.tile([G, Fc, W], F32, name="res", tag=f"res{c}")
            ress.append(res)

        def mm512(out, lhsT, rhs, cols):
            """matmul in <=512-column pieces (PSUM bank limit)."""
            of = out.rearrange("p f w -> p (f w)") if len(out.shape) == 3 else out
            rf = rhs.rearrange("p f w -> p (f w)") if len(rhs.shape) == 3 else rhs
            for lo in range(0, cols, 512):
                hi = min(lo + 512, cols)
                nc.tensor.matmul(of[:, lo:hi], lhsT, rf[:, lo:hi],
                                 start=True, stop=True)

        def chain_mm(ps, al, Fc):
            mm512(ps, bd, al, Fc * W)

        def make_block(c, b):
            """DMA + exp + beta-mask unpack for chain c, block b."""
            Fc = CHAINS[c][1]
            ft = ft_pools[c].tile([P, Fc, LB, J], BF16, name="ft", tag="ft")
            woff, qoff = (b * LB) // SEG, (b * LB) % SEG
            segs = SEGMAP[c]
            w0 = segs[0] + woff
            stride = segs[1] - segs[0] if Fc > 1 else 1
            src = f_r[:, w0:w0 + stride * (Fc - 1) + 1:stride,
                      qoff:qoff + LB, :]
            nc.sync.dma_start(out=ft, in_=src)
            ec = ec_pools[c].tile([P, Fc, LB, W], BF16, name="ec", tag="ec")
            nc.scalar.activation(
                out=ec[:, :, :, 0:J], in_=ft,
                func=mybir.ActivationFunctionType.Exp, bias=bias_a, scale=1.0)
            if b % OHQ == 0:
                oh_tiles[c] = stage_onehot(c, b)
            nc.gpsimd.tensor_mul(out=ec[:, :, :, J:W],
                                 in0=oh_tiles[c][:, b % OHQ],
                                 in1=ec[:, :, :, 0:J])
            return ec

        # ---- initial state + first blocks ----
        alphas, ecs = [], []
        for c in range(C):
            Fc = CHAINS[c][1]
            al = state_pools[c].tile([P, Fc, W], BF16, name="al", tag="al")
            nc.sync.dma_start(out=al, in_=init_ins[c][:])
            alphas.append(al)
            ecs.append(make_block(c, 0))

        for rep in range(nrep):
          for b in range(NB):
            for c in range(C):
                Fc = CHAINS[c][1]
                kind = CHAINS[c][0]
                new_ec = make_block(c, b + 1) if b + 1 < NB else None
                ec = ecs[c]
                for q in range(LB):
                    n = b * LB + q
                    if n == BI:
                        # boundary snapshot: 1/sum of state BEFORE step t_s
                        bi_ps = bps_pool.tile([G, Fc, W], F32, name="bi",
                                              tag="bps")
                        mm512(bi_ps, ones_bd, alphas[c], Fc * W)
                        ln_bi = small_pool.tile([G, Fc, W], F32,
                                                name="lnbi", tag=f"lnbi{c}")
                        nc.scalar.activation(
                            out=ln_bi[:, :, 0:J], in_=bi_ps[:, :, 0:J],
                            func=mybir.ActivationFunctionType.Ln, scale=1.0)
                        nc.scalar.activation(
                            out=ln_bi[:, :, J:W], in_=bi_ps[:, :, J:W],
                            func=mybir.ActivationFunctionType.Ln,
                            scale=float(math.exp(SBI_B)))
                        nc.vector.tensor_scalar_mul(out=accs[c], in0=ln_bi,
                                                    scalar1=-1.0)
                        if c == 0:
                            nc.vector.memset(accs[c][:, 0, :], 0.0)

                    ps = cps_pools[c].tile([P, Fc, W], F32, name="ps",
                                           tag="ps")
                    chain_mm(ps, alphas[c], Fc)
                    alphas[c] = state_pools[c].tile([P, Fc, W], BF16,
                                                    name="al", tag="al")
                    if kind == "v":
                        nc.vector.tensor_mul(out=alphas[c], in0=ps,
                                             in1=ec[:, :, q, :])
                    else:
                        sb = sbx_pools[c].tile([P, Fc, W], BF16, name="sb",
                                               tag="sb")
                        nc.scalar.activation(
                            out=sb, in_=ps,
                            func=mybir.ActivationFunctionType.Identity,
                            scale=1.0)
                        nc.gpsimd.tensor_mul(out=alphas[c], in0=sb,
                                             in1=ec[:, :, q, :])

                    if n == BI and c == 0:
                        # seg 0: exact re-init (t==0 path), both halves
                        nc.vector.tensor_scalar_mul(
                            out=alphas[c][:, 0, :], in0=ec[:, 0, q, :],
                            scalar1=astart)

                    if c == 1 and n in dbg_outs:
                        dal = small_pool.tile([P, Fc, W], F32, name="dal",
                                              tag="dal")
                        nc.vector.tensor_scalar_mul(out=dal, in0=alphas[c],
                                                    scalar1=1.0)
                        nc.sync.dma_start(out=dbg_outs[n][:], in_=dal)
                        dec = small_pool.tile([P, Fc, W], F32, name="dec",
                                              tag="dec")
                        nc.vector.tensor_scalar_mul(out=dec,
                                                    in0=ec[:, :, q, :],
                                                    scalar1=1.0)
                        nc.sync.dma_start(out=dbg_ec[n][:], in_=dec)

                    if n % RS == RPH:
                        s_ps = bps_pool.tile([G, Fc, W], F32, name="sps",
                                             tag="bps")
                        mm512(s_ps, ones_bd, alphas[c], Fc * W)
                        r_sb = small_pool.tile([G, Fc, W], F32, name="r",
                                               tag=f"r{c}")
                        nc.vector.reciprocal(out=r_sb, in_=s_ps)
                        ln_sb = small_pool.tile([G, Fc, W], F32, name="ln",
                                                tag=f"ln{c}")
                        nc.scalar.activation(
                            out=ln_sb[:, :, 0:J], in_=s_ps[:, :, 0:J],
                            func=mybir.ActivationFunctionType.Ln, scale=1.0)
                        nc.scalar.activation(
                            out=ln_sb[:, :, J:W], in_=s_ps[:, :, J:W],
                            func=mybir.ActivationFunctionType.Ln,
                            scale=float(math.exp(
                                SBU1_B if n == RPH else SBU_B)))
                        nc.vector.tensor_add(out=accs[c], in0=accs[c],
                                             in1=ln_sb)
                        rb_ps = bps_pool.tile([P, Fc, W], F32, name="rbp",
                                              tag="bps")
                        mm512(rb_ps, bcast, r_sb, Fc * W)
                        rb_sb = small_pool.tile([P, Fc, W], BF16, name="rb",
                                                tag=f"rb{c}")
                        nc.scalar.activation(
                            out=rb_sb, in_=rb_ps,
                            func=mybir.ActivationFunctionType.Identity,
                            scale=1.0)
                        nc.gpsimd.tensor_mul(
                            out=ec[:, :, q + 4, :], in0=ec[:, :, q + 4, :],
                            in1=rb_sb)
                if new_ec is not None:
                    ecs[c] = new_ec

        # ---- per-chain finalization: res = ln(fin * SC_FIN * acc) ----
        for c in range(C):
            Fc = CHAINS[c][1]
            fin_ps = bps_pool.tile([G, Fc, W], F32, name="fin", tag="bps")
            mm512(fin_ps, ones_bd, alphas[c], Fc * W)
            if S - 1 in SEGMAP[c]:
                # last segment: astop-weighted final sum overrides its slot
                fin2 = bps_pool.tile([G, W], F32, name="fin2", tag="bps")
                nc.tensor.matmul(fin2, astop, alphas[c][:, Fc - 1, :],
                                 start=True, stop=True)
            ln_fin = small_pool.tile([G, Fc, W], F32, name="lnf",
                                     tag=f"lnf{c}")
            nc.scalar.activation(out=ln_fin[:, :, 0:J],
                                 in_=fin_ps[:, :, 0:J],
                                 func=mybir.ActivationFunctionType.Ln,
                                 scale=1.0)
            nc.scalar.activation(out=ln_fin[:, :, J:W],
                                 in_=fin_ps[:, :, J:W],
                                 func=mybir.ActivationFunctionType.Ln,
                                 scale=float(math.exp(SFI_B)))
            if S - 1 in SEGMAP[c]:
                ln2 = small_pool.tile([G, W], F32, name="ln2", tag="ln2")
                nc.scalar.activation(out=ln2[:, 0:J], in_=fin2[:, 0:J],
                                     func=mybir.ActivationFunctionType.Ln,
                                     scale=1.0)
                nc.scalar.activation(out=ln2[:, J:W], in_=fin2[:, J:W],
                                     func=mybir.ActivationFunctionType.Ln,
                                     scale=float(math.exp(SFI_B)))
                nc.vector.tensor_scalar_mul(out=ln_fin[:, Fc - 1, :],
                                            in0=ln2, scalar1=1.0)
            nc.vector.tensor_add(out=ress[c], in0=accs[c], in1=ln_fin)
            if dbg_acc:
                nc.sync.dma_start(out=dbg_acc[c][:], in_=accs[c])
                nc.sync.dma_start(out=dbg_res[c][:], in_=ress[c])

        # ---- assembly: nll = sum_s res_alpha - sum_s res_beta + const ----
        tot = small_pool.tile([G, W], F32, name="tot")
        nc.gpsimd.tensor_add(out=tot, in0=ress[0][:, 0, :], in1=ress[0][:, 1, :])
        for c in range(C):
            k0 = 2 if c == 0 else 0
            for k in range(k0, CHAINS[c][1]):
                nc.gpsimd.tensor_add(out=tot, in0=tot, in1=ress[c][:, k, :])
        nll_sb = small_pool.tile([G, J], F32, name="nll_sb")
        nc.gpsimd.tensor_sub(out=nll_sb, in0=tot[:, 0:J], in1=tot[:, J:W])
        nc.gpsimd.tensor_scalar_add(out=nll_sb, in0=nll_sb,
                                    scalar1=float(NLL_CONST))
        nc.sync.dma_start(out=out_ext[:], in_=nll_sb)

    nc.finalize()
    return nc


def _host_prep(feats, tags, transitions):
    """Build per-core input maps. Layout/dtype staging only — all FLOPs on
    device except the 11x11 exp(transitions) weight build."""
    import ml_dtypes
    f32 = np.float32
    bf16 = ml_dtypes.bfloat16
    feats = np.asarray(feats, dtype=f32)
    tags_i = np.asarray(tags).astype(np.int32)
    trans = np.asarray(transitions, dtype=f32)

    def padp(a):
        out = np.zeros((P,) + a.shape[1:], dtype=a.dtype)
        out[:PL] = a
        return np.ascontiguousarray(out)

    A = np.exp(trans.astype(np.float64)).astype(f32)     # A[next, prev]
    Abody = A[:KT, :KT]
    eye = np.eye(G, dtype=f32)
    bd0 = np.kron(eye, Abody.T)
    bd = np.zeros((P, P), dtype=bf16)
    bd[:PL, :PL] = bd0.astype(bf16)
    astart = padp(np.tile(A[:KT, START], G)[:, None].astype(f32))
    astop = padp(np.kron(eye, A[STOP, :KT].reshape(KT, 1)).astype(bf16))
    ones_bd = padp(np.kron(eye, np.ones((KT, 1), f32)).astype(bf16))
    bcast = np.zeros((G, P), dtype=f32)
    bcast[:, :PL] = np.kron(eye, np.ones((1, KT), f32))

    nslots = G * J
    in_maps = []
    for core in range(NCORES):
        fb = feats[core * BL:(core + 1) * BL, :, :KT]
        tb = tags_i[core * BL:(core + 1) * BL]
        fpad = np.zeros((nslots, T, KT), dtype=f32)
        fpad[:BL] = fb
        tpad_s = np.zeros((nslots, T), dtype=np.int32)
        tpad_s[:BL] = tb
        ftime = np.zeros((nslots, TP, KT), dtype=f32)
        ftime[:, BI:BI + T] = fpad
        feats_T = padp(np.ascontiguousarray(
            ftime.reshape(G, J, TP, KT).transpose(0, 3, 2, 1)
            .reshape(PL, TP, J)).astype(bf16))
        ttime = np.zeros((nslots, TP), dtype=np.int32)
        ttime[:, BI:BI + T] = tpad_s
        ttime[:, :BI] = tpad_s[:, :1]
        tg_gj = ttime.reshape(G, J, TP)

        core_map = {
            "feats_t": feats_T,
            "bd_lhst": bd,
            "astart": astart,
            "astop": astop,
            "ones_bd": ones_bd,
            "bcast": bcast,
        }
        for c in range(C):
            Fc = CHAINS[c][1]
            segs = _seg_list(c)
            ohm = np.zeros((P, NB, Fc, LB, J), dtype=np.uint8)
            init_st = np.zeros((P, Fc, W), dtype=bf16)
            init_st[:PL, :, 0:J] = 1.0
            for k, s in enumerate(segs):
                win = tg_gj[:, :, s * SEG:s * SEG + N]    # [G, J, N]
                oh = (np.arange(KT)[:, None, None, None] ==
                      win[None]).astype(np.uint8)          # [KT, G, J, N]
                # [KT, G, J, NB, LB] -> [(G,KT), NB, LB, J]
                ohm[:PL, :, k] = (oh.reshape(KT, G, J, NB, LB)
                                  .transpose(1, 0, 3, 4, 2)
                                  .reshape(PL, NB, LB, J))
                prev = ttime[:, s * SEG - 1] if s > 0 else ttime[:, 0]
                onehot = (np.arange(KT)[:, None, None] ==
                          prev.reshape(1, G, J)).astype(f32)
                init_st[:PL, k, J:W] = (
                    onehot.transpose(1, 0, 2).reshape(PL, J).astype(bf16))
            core_map[f"onehot{c}"] = ohm
            core_map[f"init_st{c}"] = init_st
        in_maps.append(core_map)
    return in_maps


LAST_EXEC_NS = None


def kernel(feats, tags, transitions):
    global LAST_EXEC_NS
    in_maps = _host_prep(feats, tags, transitions)
    nc = _build_nc()
    trace = os.environ.get("KERNEL_TRACE") == "1"
    res = None
    for attempt in range(3):
        try:
            res = run_bass_kernel_spmd(
                nc, in_maps, list(range(NCORES)), trace=trace)
            break
        except Exception:
            if attempt == 2:
                raise
            import time as _time
            import jax as _jax
            try:
                _jax.clear_caches()
            except Exception:
                pass
            for fn in ("clear_backends",):
                try:
                    getattr(_jax.extend.backend, fn)()
                except Exception:
                    try:
                        getattr(_jax, fn)()
                    except Exception:
                        pass
            _time.sleep(5)
    LAST_EXEC_NS = res.exec_time_ns
    outs = []
    for core in range(NCORES):
        nll_parts = np.asarray(res.results[core]["nll"], dtype=np.float32)
        outs.append(nll_parts.reshape(-1)[:BL])
    return np.concatenate(outs).astype(np.float32)


if __name__ == "__main__":
    rng = np.random.default_rng(0)
    feats = rng.standard_normal((B, T, K), dtype=np.float32)
    tags = rng.integers(0, 9, size=(B, T), dtype=np.int64)
    trans = rng.random((K, K), dtype=np.float32)
    trans[START, :] = -10000.0
    trans[:, STOP] = -10000.0
    out = kernel(feats=feats, tags=tags, transitions=trans)
    print(out.shape, out[:4])


# revision 31
# speedup vs baseline: 4.3221x; 1.1177x over previous
"""CRF NLL loss kernel for Trainium2 (8 NeuronCores, batch-parallel).

Strategy (v3): time-segmented forward recursion. The per-step chain
matmul(PE) -> emission-multiply has ~430ns serial latency, so a single
T=2048 chain is latency-bound (~1ms). The CRF transfer operator is strongly
contracting (Birkhoff coefficient <= tanh(0.5) ~ 0.46/step since
log A in [0,1)), so the forward direction forgets its init in ~8 steps. We
split T into S=16 segments of 128 steps; each segment chain starts BI=8
steps early (alpha: uniform init; gold/beta: exact one-hot at the known
gold tag) and per-segment results are stitched by a telescoping boundary
correction validated in proto.py:

  logX = sum_s ln( fin_sum_s * prod(rescale sums after start) / sum@start )

Chains fuse F segments per instruction and run staggered. GPSIMD cannot
touch PSUM on real HW, so the PSUM crossing (matmul output -> SBUF) runs on
DVE ("v" chains: fused multiply) or ACT ("a" chains: copy, then Pool does
the SBUF multiply). The gold half's masked emission ec_beta = onehot *
ec_alpha is built by Pool from host-packed one-hot BITS:
(packed & (1<<q)) * ec_alpha — the stray 2^q factor is a compile-time
constant absorbed into the final nll offset. Scale accumulators are kept in
product form (one Ln per chain at the end) to avoid ACT function-table
swaps; per-half centering immediates keep everything in f32/bf16 range.
"""
import math
import os
import sys

import numpy as np

sys.path.insert(0, "/opt/trn_rl_repo")

from contextlib import ExitStack

import concourse.bacc as bacc
import concourse.bass as bass
import concourse.tile as tile
from concourse import mybir
from concourse.bass_utils import run_bass_kernel_spmd

# problem constants (hardcoded per spec)
B, T, K = 4096, 2048, 11
START, STOP = 10, 9
NCORES = 8
BL = B // NCORES          # 512 sentences per core
G, KT, J = 14, 9, 37      # groups x body-tags x sentences-per-group (518 slots)
P = 128
PL = G * KT               # live partitions
W = 2 * J                 # 74 free per segment: [alpha | beta]

# segmentation
S = 16                    # segments
SEG = T // S              # 128 official steps per segment
BI = 8                    # burn-in steps
N = SEG + BI              # chain steps
LB = 8                    # steps per ec/ft block
NB = N // LB              # blocks per chain
RS = 16                   # rescale cadence (beta ~e^-2.7/step)
RPH = 11                  # rescale at n in {11,27,...,123}
NBUND = 8                 # bundles per chain
TP = 17 * SEG             # padded time length (BI junk + T + tail junk)
C0A = 3.2                 # per-step log recentering (shared by both halves)

# chain layout: list of (cross_engine, segments). "v": DVE fused multiply;
# "a": ACT copy + Pool multiply. Each chain's segments must be equally
# spaced (the block DMA uses one strided access pattern).
SEGMAP = [[0, 2, 4, 6, 8, 10, 12, 14], [1, 3, 5, 7, 9, 11, 13, 15]]
KINDS = ["v", "v"]
C = len(SEGMAP)
CHAINS = [(KINDS[c], len(SEGMAP[c])) for c in range(C)]
assert sorted(s for segs in SEGMAP for s in segs) == list(range(S))
for segs in SEGMAP:
    assert len(set(np.diff(segs))) <= 1

# Ln LUT inputs are pre-scaled to land near 1 (the LUT loses precision at
# extreme magnitudes). The alpha half stays near 1 (scale 1.0); the beta
# (gold one-hot) half decays ~e^-2.7/step: its sums sit near e^-21 at the
# snapshot (~8 steps), e^-32 at the first bundle, e^-42 at later bundles
# (16-step windows) and e^-32 at the final read (12 steps). The ln(scale)
# offsets are per-column-class constants folded into NLL_CONST.
SBI_B, SBU1_B, SBU_B, SFI_B = 21.0, 32.0, 42.0, 32.0
BETA_LN_SUM = SBU1_B + (NBUND - 1) * SBU_B + SFI_B
# every beta segment column: res_b = true + (-SBI_B + BETA_LN_SUM)
# (segment 0's snapshot slot is memset to 1.0, so its -SBI_B is constant too)
NLL_CONST = S * (-SBI_B + BETA_LN_SUM)

F32 = mybir.dt.float32
BF16 = mybir.dt.bfloat16
U8 = mybir.dt.uint8


def _seg_list(c):
    return SEGMAP[c]


def _build_nc(nrep=1):
    nc = bacc.Bacc()
    f_in = nc.declare_dram_parameter("feats_t", [P, TP, J], BF16, isOutput=False)
    oh_ins = [nc.declare_dram_parameter(f"onehot{c}",
                                        [P, NB, CHAINS[c][1], LB, J],
                                        U8, isOutput=False) for c in range(C)]
    bd_in = nc.declare_dram_parameter("bd_lhst", [P, P], BF16, isOutput=False)
    astart_in = nc.declare_dram_parameter("astart", [P, 1], F32, isOutput=False)
    astop_in = nc.declare_dram_parameter("astop", [P, G], BF16, isOutput=False)
    ones_in = nc.declare_dram_parameter("ones_bd", [P, G], BF16, isOutput=False)
    bcast_in = nc.declare_dram_parameter("bcast", [G, P], F32, isOutput=False)
    init_ins = [nc.declare_dram_parameter(f"init_st{c}", [P, CHAINS[c][1], W],
                                          BF16, isOutput=False)
                for c in range(C)]
    out_ext = nc.declare_dram_parameter("nll", [G, J], F32, isOutput=True)
    dbg_steps = [0, 1, 2, 7, 8, 9, 19, 20] if os.environ.get("KDBG") else []
    dbg_outs = {n: nc.declare_dram_parameter(f"dbg{n}", [P, CHAINS[1][1], W],
                                             F32, isOutput=True)
                for n in dbg_steps}
    dbg_ec = {n: nc.declare_dram_parameter(f"dbgec{n}", [P, CHAINS[1][1], W],
                                           F32, isOutput=True)
              for n in dbg_steps}
    dbg_acc = [nc.declare_dram_parameter(f"dbgacc{c}", [G, CHAINS[c][1], W],
                                         F32, isOutput=True)
               for c in range(C)] if dbg_steps else []
    dbg_res = [nc.declare_dram_parameter(f"dbgres{c}", [G, CHAINS[c][1], W],
                                         F32, isOutput=True)
               for c in range(C)] if dbg_steps else []

    f_r = f_in.rearrange("p (w q) j -> p w q j", q=SEG)   # [P, 17, 128, J]

    with tile.TileContext(nc) as tc, ExitStack() as ctx:
        consts = ctx.enter_context(tc.tile_pool(name="consts", bufs=1))
        ft_pools = [ctx.enter_context(tc.tile_pool(name=f"ft{c}", bufs=2))
                    for c in range(C)]
        ec_pools = [ctx.enter_context(tc.tile_pool(name=f"ec{c}", bufs=2))
                    for c in range(C)]
        state_pools = [ctx.enter_context(tc.tile_pool(name=f"st{c}", bufs=3))
                       for c in range(C)]
        sbx_pools = [ctx.enter_context(tc.tile_pool(name=f"sbx{c}", bufs=2))
                     for c in range(C)]
        small_pool = ctx.enter_context(tc.tile_pool(name="small", bufs=2))
        cps_pools = [ctx.enter_context(
            tc.tile_pool(name=f"cpsum{c}", bufs=1, space="PSUM"))
            for c in range(C)]
        bps_pool = ctx.enter_context(
            tc.tile_pool(name="bpsum", bufs=2, space="PSUM"))

        # ---- constants ----
        bd = consts.tile([P, P], BF16, name="bd")
        nc.sync.dma_start(out=bd, in_=bd_in[:])
        astart = consts.tile([P, 1], F32, name="astart")
        nc.sync.dma_start(out=astart, in_=astart_in[:])
        astop = consts.tile([P, G], BF16, name="astop")
        nc.sync.dma_start(out=astop, in_=astop_in[:])
        ones_bd = consts.tile([P, G], BF16, name="ones_bd")
        nc.sync.dma_start(out=ones_bd, in_=ones_in[:])
        bcast = consts.tile([G, P], F32, name="bcast")
        nc.sync.dma_start(out=bcast, in_=bcast_in[:])
        bias_a = consts.tile([P, 1], F32, name="bias_a")
        nc.vector.memset(bias_a, -C0A)
        oh_pools = [ctx.enter_context(tc.tile_pool(name=f"oh{c}", bufs=2))
                    for c in range(C)]
        OHQ = 3                           # blocks per one-hot staging DMA
        oh_tiles = [None] * C

        def stage_onehot(c, b):
            Fc = CHAINS[c][1]
            nblk = min(OHQ, NB - b)
            oh = oh_pools[c].tile([P, OHQ, Fc, LB, J], U8, name="oh",
                                  tag="oh")
            nc.sync.dma_start(out=oh[:, 0:nblk], in_=oh_ins[c][:, b:b + nblk])
            return oh

        NST = NBUND + 2               # stash slots: bundles, snapshot, fin
        stashes, ress = [], []
        for c in range(C):
            Fc = CHAINS[c][1]
            st = consts.tile([G, NST, Fc, W], F32, name="st", tag=f"st{c}")
            stashes.append(st)
            res = consts.tile([G, Fc, W], F32, name="res", tag=f"res{c}")
            ress.append(res)

        def mm512(out, lhsT, rhs, cols):
            """matmul in <=512-column pieces (PSUM bank limit)."""
            of = out.rearrange("p f w -> p (f w)") if len(out.shape) == 3 else out
            rf = rhs.rearrange("p f w -> p (f w)") if len(rhs.shape) == 3 else rhs
            for lo in range(0, cols, 512):
                hi = min(lo + 512, cols)
                nc.tensor.matmul(of[:, lo:hi], lhsT, rf[:, lo:hi],
                                 start=True, stop=True)

        def chain_mm(ps, al, Fc):
            mm512(ps, bd, al, Fc * W)

        def make_block(c, b):
            """DMA + exp + beta-mask unpack for chain c, block b."""
            Fc = CHAINS[c][1]
            ft = ft_pools[c].tile([P, Fc, LB, J], BF16, name="ft", tag="ft")
            woff, qoff = (b * LB) // SEG, (b * LB) % SEG
            segs = SEGMAP[c]
            w0 = segs[0] + woff
            stride = segs[1] - segs[0] if Fc > 1 else 1
            src = f_r[:, w0:w0 + stride * (Fc - 1) + 1:stride,
                      qoff:qoff + LB, :]
            nc.sync.dma_start(out=ft, in_=src)
            ec = ec_pools[c].tile([P, Fc, LB, W], BF16, name="ec", tag="ec")
            nc.scalar.activation(
                out=ec[:, :, :, 0:J], in_=ft,
                func=mybir.ActivationFunctionType.Exp, bias=bias_a, scale=1.0)
            if b % OHQ == 0:
                oh_tiles[c] = stage_onehot(c, b)
            nc.gpsimd.tensor_mul(out=ec[:, :, :, J:W],
                                 in0=oh_tiles[c][:, b % OHQ],
                                 in1=ec[:, :, :, 0:J])
            return ec

        # ---- initial state + first blocks ----
        alphas, ecs = [], []
        for c in range(C):
            Fc = CHAINS[c][1]
            al = state_pools[c].tile([P, Fc, W], BF16, name="al", tag="al")
            nc.sync.dma_start(out=al, in_=init_ins[c][:])
            alphas.append(al)
            ecs.append(make_block(c, 0))

        for rep in range(nrep):
          for b in range(NB):
            for c in range(C):
                Fc = CHAINS[c][1]
                kind = CHAINS[c][0]
                new_ec = make_block(c, b + 1) if b + 1 < NB else None
                ec = ecs[c]
                for q in range(LB):
                    n = b * LB + q
                    if n == BI:
                        # boundary snapshot: 1/sum of state BEFORE step t_s
                        bi_ps = bps_pool.tile([G, Fc, W], F32, name="bi",
                                              tag="bps")
                        mm512(bi_ps, ones_bd, alphas[c], Fc * W)
                        nc.scalar.activation(
                            out=stashes[c][:, NBUND], in_=bi_ps,
                            func=mybir.ActivationFunctionType.Identity,
                            scale=1.0)
                        if c == 0:
                            nc.vector.memset(stashes[c][:, NBUND, 0, :], 1.0)

                    ps = cps_pools[c].tile([P, Fc, W], F32, name="ps",
                                           tag="ps")
                    chain_mm(ps, alphas[c], Fc)
                    alphas[c] = state_pools[c].tile([P, Fc, W], BF16,
                                                    name="al", tag="al")
                    if kind == "v":
                        nc.vector.tensor_mul(out=alphas[c], in0=ps,
                                             in1=ec[:, :, q, :])
                    else:
                        sb = sbx_pools[c].tile([P, Fc, W], BF16, name="sb",
                                               tag="sb")
                        nc.scalar.activation(
                            out=sb, in_=ps,
                            func=mybir.ActivationFunctionType.Identity,
                            scale=1.0)
                        nc.gpsimd.tensor_mul(out=alphas[c], in0=sb,
                                             in1=ec[:, :, q, :])

                    if n == BI and c == 0:
                        # seg 0: exact re-init (t==0 path), both halves
                        nc.vector.tensor_scalar_mul(
                            out=alphas[c][:, 0, :], in0=ec[:, 0, q, :],
                            scalar1=astart)

                    if c == 1 and n in dbg_outs:
                        dal = small_pool.tile([P, Fc, W], F32, name="dal",
                                              tag="dal")
                        nc.vector.tensor_scalar_mul(out=dal, in0=alphas[c],
                                                    scalar1=1.0)
                        nc.sync.dma_start(out=dbg_outs[n][:], in_=dal)
                        dec = small_pool.tile([P, Fc, W], F32, name="dec",
                                              tag="dec")
                        nc.vector.tensor_scalar_mul(out=dec,
                                                    in0=ec[:, :, q, :],
                                                    scalar1=1.0)
                        nc.sync.dma_start(out=dbg_ec[n][:], in_=dec)

                    if n % RS == RPH:
                        s_ps = bps_pool.tile([G, Fc, W], F32, name="sps",
                                             tag="bps")
                        mm512(s_ps, ones_bd, alphas[c], Fc * W)
                        r_sb = small_pool.tile([G, Fc, W], F32, name="r",
                                               tag=f"r{c}")
                        nc.vector.reciprocal(out=r_sb, in_=s_ps)
                        nc.scalar.activation(
                            out=stashes[c][:, n // RS], in_=s_ps,
                            func=mybir.ActivationFunctionType.Identity,
                            scale=1.0)
                        rb_ps = bps_pool.tile([P, Fc, W], F32, name="rbp",
                                              tag="bps")
                        mm512(rb_ps, bcast, r_sb, Fc * W)
                        rb_sb = small_pool.tile([P, Fc, W], BF16, name="rb",
                                                tag=f"rb{c}")
                        nc.scalar.activation(
                            out=rb_sb, in_=rb_ps,
                            func=mybir.ActivationFunctionType.Identity,
                            scale=1.0)
                        nc.gpsimd.tensor_mul(
                            out=ec[:, :, q + 4, :], in0=ec[:, :, q + 4, :],
                            in1=rb_sb)
                if new_ec is not None:
                    ecs[c] = new_ec

        # ---- per-chain finalization via the stash ----
        for c in range(C):
            Fc = CHAINS[c][1]
            fin_ps = bps_pool.tile([G, Fc, W], F32, name="fin", tag="bps")
            mm512(fin_ps, ones_bd, alphas[c], Fc * W)
            nc.scalar.activation(out=stashes[c][:, NBUND + 1], in_=fin_ps,
                                 func=mybir.ActivationFunctionType.Identity,
                                 scale=1.0)
            if S - 1 in SEGMAP[c]:
                # last segment: astop-weighted final sum overrides its slot
                fin2 = bps_pool.tile([G, W], F32, name="fin2", tag="bps")
                nc.tensor.matmul(fin2, astop, alphas[c][:, Fc - 1, :],
                                 start=True, stop=True)
                nc.scalar.activation(
                    out=stashes[c][:, NBUND + 1, Fc - 1, :], in_=fin2,
                    func=mybir.ActivationFunctionType.Identity, scale=1.0)
        # batched Lns (one activation-table load), then Pool combines
        for c in range(C):
            Fc = CHAINS[c][1]
            st = stashes[c]
            lnst = st  # Ln applied in place (ACT reads then writes each elem)
            nc.scalar.activation(out=lnst[:, :, :, 0:J], in_=st[:, :, :, 0:J],
                                 func=mybir.ActivationFunctionType.Ln,
                                 scale=1.0)
            for sl, sc in [((0,), SBU1_B), (tuple(range(1, NBUND)), SBU_B),
                           ((NBUND,), SBI_B), ((NBUND + 1,), SFI_B)]:
                lo, hi = sl[0], sl[-1] + 1
                nc.scalar.activation(
                    out=lnst[:, lo:hi, :, J:W], in_=st[:, lo:hi, :, J:W],
                    func=mybir.ActivationFunctionType.Ln,
                    scale=float(math.exp(sc)))
            acc = small_pool.tile([G, Fc, W], F32, name="acc", tag=f"acc{c}")
            nc.gpsimd.tensor_add(out=acc, in0=lnst[:, 0], in1=lnst[:, 1])
            for i in range(2, NBUND):
                nc.gpsimd.tensor_add(out=acc, in0=acc, in1=lnst[:, i])
            nc.gpsimd.tensor_add(out=acc, in0=acc, in1=lnst[:, NBUND + 1])
            nc.gpsimd.tensor_sub(out=ress[c], in0=acc, in1=lnst[:, NBUND])

        # ---- assembly: nll = sum_s res_alpha - sum_s res_beta + const ----
        tot = small_pool.tile([G, W], F32, name="tot")
        nc.gpsimd.tensor_add(out=tot, in0=ress[0][:, 0, :], in1=ress[0][:, 1, :])
        for c in range(C):
            k0 = 2 if c == 0 else 0
            for k in range(k0, CHAINS[c][1]):
                nc.gpsimd.tensor_add(out=tot, in0=tot, in1=ress[c][:, k, :])
        nll_sb = small_pool.tile([G, J], F32, name="nll_sb")
        nc.gpsimd.tensor_sub(out=nll_sb, in0=tot[:, 0:J], in1=tot[:, J:W])
        nc.gpsimd.tensor_scalar_add(out=nll_sb, in0=nll_sb,
                                    scalar1=float(NLL_CONST))
        nc.sync.dma_start(out=out_ext[:], in_=nll_sb)

    nc.finalize()
    return nc


def _host_prep(feats, tags, transitions):
    """Build per-core input maps. Layout/dtype staging only — all FLOPs on
    device except the 11x11 exp(transitions) weight build."""
    import ml_dtypes
    f32 = np.float32
    bf16 = ml_dtypes.bfloat16
    feats = np.asarray(feats, dtype=f32)
    tags_i = np.asarray(tags).astype(np.int32)
    trans = np.asarray(transitions, dtype=f32)

    def padp(a):
        out = np.zeros((P,) + a.shape[1:], dtype=a.dtype)
        out[:PL] = a
        return np.ascontiguousarray(out)

    A = np.exp(trans.astype(np.float64)).astype(f32)     # A[next, prev]
    Abody = A[:KT, :KT]
    eye = np.eye(G, dtype=f32)
    bd0 = np.kron(eye, Abody.T)
    bd = np.zeros((P, P), dtype=bf16)
    bd[:PL, :PL] = bd0.astype(bf16)
    astart = padp(np.tile(A[:KT, START], G)[:, None].astype(f32))
    astop = padp(np.kron(eye, A[STOP, :KT].reshape(KT, 1)).astype(bf16))
    ones_bd = padp(np.kron(eye, np.ones((KT, 1), f32)).astype(bf16))
    bcast = np.zeros((G, P), dtype=f32)
    bcast[:, :PL] = np.kron(eye, np.ones((1, KT), f32))

    nslots = G * J
    in_maps = []
    for core in range(NCORES):
        fb = feats[core * BL:(core + 1) * BL, :, :KT]
        tb = tags_i[core * BL:(core + 1) * BL]
        fpad = np.zeros((nslots, T, KT), dtype=f32)
        fpad[:BL] = fb
        tpad_s = np.zeros((nslots, T), dtype=np.int32)
        tpad_s[:BL] = tb
        ftime = np.zeros((nslots, TP, KT), dtype=f32)
        ftime[:, BI:BI + T] = fpad
        feats_T = padp(np.ascontiguousarray(
            ftime.reshape(G, J, TP, KT).transpose(0, 3, 2, 1)
            .reshape(PL, TP, J)).astype(bf16))
        ttime = np.zeros((nslots, TP), dtype=np.int32)
        ttime[:, BI:BI + T] = tpad_s
        ttime[:, :BI] = tpad_s[:, :1]
        tg_gj = ttime.reshape(G, J, TP)

        core_map = {
            "feats_t": feats_T,
            "bd_lhst": bd,
            "astart": astart,
            "astop": astop,
            "ones_bd": ones_bd,
            "bcast": bcast,
        }
        for c in range(C):
            Fc = CHAINS[c][1]
            segs = _seg_list(c)
            ohm = np.zeros((P, NB, Fc, LB, J), dtype=np.uint8)
            init_st = np.zeros((P, Fc, W), dtype=bf16)
            init_st[:PL, :, 0:J] = 1.0
            for k, s in enumerate(segs):
                win = tg_gj[:, :, s * SEG:s * SEG + N]    # [G, J, N]
                oh = (np.arange(KT)[:, None, None, None] ==
                      win[None]).astype(np.uint8)          # [KT, G, J, N]
                # [KT, G, J, NB, LB] -> [(G,KT), NB, LB, J]
                ohm[:PL, :, k] = (oh.reshape(KT, G, J, NB, LB)
                                  .transpose(1, 0, 3, 4, 2)
                                  .reshape(PL, NB, LB, J))
                prev = ttime[:, s * SEG - 1] if s > 0 else ttime[:, 0]
                onehot = (np.arange(KT)[:, None, None] ==
                          prev.reshape(1, G, J)).astype(f32)
                init_st[:PL, k, J:W] = (
                    onehot.transpose(1, 0, 2).reshape(PL, J).astype(bf16))
            core_map[f"onehot{c}"] = ohm
            core_map[f"init_st{c}"] = init_st
        in_maps.append(core_map)
    return in_maps


LAST_EXEC_NS = None


def kernel(feats, tags, transitions):
    global LAST_EXEC_NS
    in_maps = _host_prep(feats, tags, transitions)
    nc = _build_nc()
    trace = os.environ.get("KERNEL_TRACE") == "1"
    res = None
    for attempt in range(3):
        try:
            res = run_bass_kernel_spmd(
                nc, in_maps, list(range(NCORES)), trace=trace)
            break
        except Exception:
            if attempt == 2:
                raise
            import time as _time
            import jax as _jax
            try:
                _jax.clear_caches()
            except Exception:
                pass
            for fn in ("clear_backends",):
                try:
                    getattr(_jax.extend.backend, fn)()
                except Exception:
                    try:
                        getattr(_jax, fn)()
                    except Exception:
                        pass
            _time.sleep(5)
    LAST_EXEC_NS = res.exec_time_ns
    outs = []
    for core in range(NCORES):
        nll_parts = np.asarray(res.results[core]["nll"], dtype=np.float32)
        outs.append(nll_parts.reshape(-1)[:BL])
    return np.concatenate(outs).astype(np.float32)


if __name__ == "__main__":
    rng = np.random.default_rng(0)
    feats = rng.standard_normal((B, T, K), dtype=np.float32)
    tags = rng.integers(0, 9, size=(B, T), dtype=np.int64)
    trans = rng.random((K, K), dtype=np.float32)
    trans[START, :] = -10000.0
    trans[:, STOP] = -10000.0
    out = kernel(feats=feats, tags=tags, transitions=trans)
    print(out.shape, out[:4])


# revision 32
# speedup vs baseline: 4.4047x; 1.0191x over previous
"""CRF NLL loss kernel for Trainium2 (8 NeuronCores, batch-parallel).

Strategy (v3): time-segmented forward recursion. The per-step chain
matmul(PE) -> emission-multiply has ~430ns serial latency, so a single
T=2048 chain is latency-bound (~1ms). The CRF transfer operator is strongly
contracting (Birkhoff coefficient <= tanh(0.5) ~ 0.46/step since
log A in [0,1)), so the forward direction forgets its init in ~8 steps. We
split T into S=16 segments of 128 steps; each segment chain starts BI=8
steps early (alpha: uniform init; gold/beta: exact one-hot at the known
gold tag) and per-segment results are stitched by a telescoping boundary
correction validated in proto.py:

  logX = sum_s ln( fin_sum_s * prod(rescale sums after start) / sum@start )

Chains fuse F segments per instruction and run staggered. GPSIMD cannot
touch PSUM on real HW, so the PSUM crossing (matmul output -> SBUF) runs on
DVE ("v" chains: fused multiply) or ACT ("a" chains: copy, then Pool does
the SBUF multiply). The gold half's masked emission ec_beta = onehot *
ec_alpha is built by Pool from host-packed one-hot BITS:
(packed & (1<<q)) * ec_alpha — the stray 2^q factor is a compile-time
constant absorbed into the final nll offset. Scale accumulators are kept in
product form (one Ln per chain at the end) to avoid ACT function-table
swaps; per-half centering immediates keep everything in f32/bf16 range.
"""
import math
import os
import sys

import numpy as np

sys.path.insert(0, "/opt/trn_rl_repo")

from contextlib import ExitStack

import concourse.bacc as bacc
import concourse.bass as bass
import concourse.tile as tile
from concourse import mybir
from concourse.bass_utils import run_bass_kernel_spmd

# problem constants (hardcoded per spec)
B, T, K = 4096, 2048, 11
START, STOP = 10, 9
NCORES = 8
BL = B // NCORES          # 512 sentences per core
G, KT, J = 14, 9, 37      # groups x body-tags x sentences-per-group (518 slots)
P = 128
PL = G * KT               # live partitions
W = 2 * J                 # 74 free per segment: [alpha | beta]

# segmentation
S = 16                    # segments
SEG = T // S              # 128 official steps per segment
BI = 4                    # burn-in steps
N = SEG + BI              # chain steps
LB = 4                    # steps per ec/ft block
NB = N // LB              # blocks per chain
RS = 16                   # rescale cadence (beta ~e^-2.7/step)
RPH = 11                  # rescale at n in {11,27,...,123}
NBUND = 8                 # bundles per chain
TP = 17 * SEG             # padded time length (BI junk + T + tail junk)
C0A = 3.2                 # per-step log recentering (shared by both halves)

# chain layout: list of (cross_engine, segments). "v": DVE fused multiply;
# "a": ACT copy + Pool multiply. Each chain's segments must be equally
# spaced (the block DMA uses one strided access pattern).
SEGMAP = [[0, 2, 4, 6, 8, 10, 12, 14], [1, 3, 5, 7, 9, 11, 13, 15]]
KINDS = ["v", "v"]
C = len(SEGMAP)
CHAINS = [(KINDS[c], len(SEGMAP[c])) for c in range(C)]
assert sorted(s for segs in SEGMAP for s in segs) == list(range(S))
for segs in SEGMAP:
    assert len(set(np.diff(segs))) <= 1

# Ln LUT inputs are pre-scaled to land near 1 (the LUT loses precision at
# extreme magnitudes). The alpha half stays near 1 (scale 1.0); the beta
# (gold one-hot) half decays ~e^-2.7/step: its sums sit near e^-21 at the
# snapshot (~8 steps), e^-32 at the first bundle, e^-42 at later bundles
# (16-step windows) and e^-32 at the final read (12 steps). The ln(scale)
# offsets are per-column-class constants folded into NLL_CONST.
SBI_B, SBU1_B, SBU_B, SFI_B = 13.0, 32.0, 42.0, 32.0
BETA_LN_SUM = SBU1_B + (NBUND - 1) * SBU_B + SFI_B
# every beta segment column: res_b = true + (-SBI_B + BETA_LN_SUM)
# (segment 0's snapshot slot is memset to 1.0, so its -SBI_B is constant too)
NLL_CONST = S * (-SBI_B + BETA_LN_SUM)

F32 = mybir.dt.float32
BF16 = mybir.dt.bfloat16
U8 = mybir.dt.uint8


def _seg_list(c):
    return SEGMAP[c]


def _build_nc(nrep=1):
    nc = bacc.Bacc()
    f_in = nc.declare_dram_parameter("feats_t", [P, TP, J], BF16, isOutput=False)
    oh_ins = [nc.declare_dram_parameter(f"onehot{c}",
                                        [P, NB, CHAINS[c][1], LB, J],
                                        U8, isOutput=False) for c in range(C)]
    bd_in = nc.declare_dram_parameter("bd_lhst", [P, P], BF16, isOutput=False)
    astart_in = nc.declare_dram_parameter("astart", [P, 1], F32, isOutput=False)
    astop_in = nc.declare_dram_parameter("astop", [P, G], BF16, isOutput=False)
    ones_in = nc.declare_dram_parameter("ones_bd", [P, G], BF16, isOutput=False)
    bcast_in = nc.declare_dram_parameter("bcast", [G, P], F32, isOutput=False)
    init_ins = [nc.declare_dram_parameter(f"init_st{c}", [P, CHAINS[c][1], W],
                                          BF16, isOutput=False)
                for c in range(C)]
    out_ext = nc.declare_dram_parameter("nll", [G, J], F32, isOutput=True)
    dbg_steps = [0, 1, 2, 7, 8, 9, 19, 20] if os.environ.get("KDBG") else []
    dbg_outs = {n: nc.declare_dram_parameter(f"dbg{n}", [P, CHAINS[1][1], W],
                                             F32, isOutput=True)
                for n in dbg_steps}
    dbg_ec = {n: nc.declare_dram_parameter(f"dbgec{n}", [P, CHAINS[1][1], W],
                                           F32, isOutput=True)
              for n in dbg_steps}
    dbg_acc = [nc.declare_dram_parameter(f"dbgacc{c}", [G, CHAINS[c][1], W],
                                         F32, isOutput=True)
               for c in range(C)] if dbg_steps else []
    dbg_res = [nc.declare_dram_parameter(f"dbgres{c}", [G, CHAINS[c][1], W],
                                         F32, isOutput=True)
               for c in range(C)] if dbg_steps else []

    f_r = f_in.rearrange("p (w q) j -> p w q j", q=SEG)   # [P, 17, 128, J]

    with tile.TileContext(nc) as tc, ExitStack() as ctx:
        consts = ctx.enter_context(tc.tile_pool(name="consts", bufs=1))
        ft_pools = [ctx.enter_context(tc.tile_pool(name=f"ft{c}", bufs=2))
                    for c in range(C)]
        ec_pools = [ctx.enter_context(tc.tile_pool(name=f"ec{c}", bufs=2))
                    for c in range(C)]
        state_pools = [ctx.enter_context(tc.tile_pool(name=f"st{c}", bufs=3))
                       for c in range(C)]
        sbx_pools = [ctx.enter_context(tc.tile_pool(name=f"sbx{c}", bufs=2))
                     for c in range(C)]
        small_pool = ctx.enter_context(tc.tile_pool(name="small", bufs=2))
        cps_pools = [ctx.enter_context(
            tc.tile_pool(name=f"cpsum{c}", bufs=1, space="PSUM"))
            for c in range(C)]
        bps_pool = ctx.enter_context(
            tc.tile_pool(name="bpsum", bufs=2, space="PSUM"))

        # ---- constants ----
        bd = consts.tile([P, P], BF16, name="bd")
        nc.sync.dma_start(out=bd, in_=bd_in[:])
        astart = consts.tile([P, 1], F32, name="astart")
        nc.sync.dma_start(out=astart, in_=astart_in[:])
        astop = consts.tile([P, G], BF16, name="astop")
        nc.sync.dma_start(out=astop, in_=astop_in[:])
        ones_bd = consts.tile([P, G], BF16, name="ones_bd")
        nc.sync.dma_start(out=ones_bd, in_=ones_in[:])
        bcast = consts.tile([G, P], F32, name="bcast")
        nc.sync.dma_start(out=bcast, in_=bcast_in[:])
        bias_a = consts.tile([P, 1], F32, name="bias_a")
        nc.vector.memset(bias_a, -C0A)
        oh_pools = [ctx.enter_context(tc.tile_pool(name=f"oh{c}", bufs=2))
                    for c in range(C)]
        OHQ = 3                           # blocks per one-hot staging DMA
        oh_tiles = [None] * C

        def stage_onehot(c, b):
            Fc = CHAINS[c][1]
            nblk = min(OHQ, NB - b)
            oh = oh_pools[c].tile([P, OHQ, Fc, LB, J], U8, name="oh",
                                  tag="oh")
            nc.sync.dma_start(out=oh[:, 0:nblk], in_=oh_ins[c][:, b:b + nblk])
            return oh

        NST = NBUND + 2               # stash slots: bundles, snapshot, fin
        stashes, ress = [], []
        for c in range(C):
            Fc = CHAINS[c][1]
            st = consts.tile([G, NST, Fc, W], F32, name="st", tag=f"st{c}")
            stashes.append(st)
            res = consts.tile([G, Fc, W], F32, name="res", tag=f"res{c}")
            ress.append(res)

        def mm512(out, lhsT, rhs, cols):
            """matmul in <=512-column pieces (PSUM bank limit)."""
            of = out.rearrange("p f w -> p (f w)") if len(out.shape) == 3 else out
            rf = rhs.rearrange("p f w -> p (f w)") if len(rhs.shape) == 3 else rhs
            for lo in range(0, cols, 512):
                hi = min(lo + 512, cols)
                nc.tensor.matmul(of[:, lo:hi], lhsT, rf[:, lo:hi],
                                 start=True, stop=True)

        def chain_mm(ps, al, Fc):
            mm512(ps, bd, al, Fc * W)

        def make_block(c, b):
            """DMA + exp + beta-mask unpack for chain c, block b."""
            Fc = CHAINS[c][1]
            ft = ft_pools[c].tile([P, Fc, LB, J], BF16, name="ft", tag="ft")
            woff, qoff = (b * LB) // SEG, (b * LB) % SEG
            segs = SEGMAP[c]
            w0 = segs[0] + woff
            stride = segs[1] - segs[0] if Fc > 1 else 1
            src = f_r[:, w0:w0 + stride * (Fc - 1) + 1:stride,
                      qoff:qoff + LB, :]
            nc.sync.dma_start(out=ft, in_=src)
            ec = ec_pools[c].tile([P, Fc, LB, W], BF16, name="ec", tag="ec")
            nc.scalar.activation(
                out=ec[:, :, :, 0:J], in_=ft,
                func=mybir.ActivationFunctionType.Exp, bias=bias_a, scale=1.0)
            if b % OHQ == 0:
                oh_tiles[c] = stage_onehot(c, b)
            nc.gpsimd.tensor_mul(out=ec[:, :, :, J:W],
                                 in0=oh_tiles[c][:, b % OHQ],
                                 in1=ec[:, :, :, 0:J])
            return ec

        # ---- initial state + first blocks ----
        alphas, ecs = [], []
        for c in range(C):
            Fc = CHAINS[c][1]
            al = state_pools[c].tile([P, Fc, W], BF16, name="al", tag="al")
            nc.sync.dma_start(out=al, in_=init_ins[c][:])
            alphas.append(al)
            ecs.append(make_block(c, 0))

        for rep in range(nrep):
          for b in range(NB):
            for c in range(C):
                Fc = CHAINS[c][1]
                kind = CHAINS[c][0]
                new_ec = make_block(c, b + 1) if b + 1 < NB else None
                ec = ecs[c]
                for q in range(LB):
                    n = b * LB + q
                    if n == BI:
                        # boundary snapshot: 1/sum of state BEFORE step t_s
                        bi_ps = bps_pool.tile([G, Fc, W], F32, name="bi",
                                              tag="bps")
                        mm512(bi_ps, ones_bd, alphas[c], Fc * W)
                        nc.scalar.activation(
                            out=stashes[c][:, NBUND], in_=bi_ps,
                            func=mybir.ActivationFunctionType.Identity,
                            scale=1.0)
                        if c == 0:
                            nc.vector.memset(stashes[c][:, NBUND, 0, :], 1.0)

                    ps = cps_pools[c].tile([P, Fc, W], F32, name="ps",
                                           tag="ps")
                    chain_mm(ps, alphas[c], Fc)
                    alphas[c] = state_pools[c].tile([P, Fc, W], BF16,
                                                    name="al", tag="al")
                    if kind == "v":
                        nc.vector.tensor_mul(out=alphas[c], in0=ps,
                                             in1=ec[:, :, q, :])
                    else:
                        sb = sbx_pools[c].tile([P, Fc, W], BF16, name="sb",
                                               tag="sb")
                        nc.scalar.activation(
                            out=sb, in_=ps,
                            func=mybir.ActivationFunctionType.Identity,
                            scale=1.0)
                        nc.gpsimd.tensor_mul(out=alphas[c], in0=sb,
                                             in1=ec[:, :, q, :])

                    if n == BI and c == 0:
                        # seg 0: exact re-init (t==0 path), both halves
                        nc.vector.tensor_scalar_mul(
                            out=alphas[c][:, 0, :], in0=ec[:, 0, q, :],
                            scalar1=astart)

                    if c == 1 and n in dbg_outs:
                        dal = small_pool.tile([P, Fc, W], F32, name="dal",
                                              tag="dal")
                        nc.vector.tensor_scalar_mul(out=dal, in0=alphas[c],
                                                    scalar1=1.0)
                        nc.sync.dma_start(out=dbg_outs[n][:], in_=dal)
                        dec = small_pool.tile([P, Fc, W], F32, name="dec",
                                              tag="dec")
                        nc.vector.tensor_scalar_mul(out=dec,
                                                    in0=ec[:, :, q, :],
                                                    scalar1=1.0)
                        nc.sync.dma_start(out=dbg_ec[n][:], in_=dec)

                    if n % RS == RPH:
                        s_ps = bps_pool.tile([G, Fc, W], F32, name="sps",
                                             tag="bps")
                        mm512(s_ps, ones_bd, alphas[c], Fc * W)
                        r_sb = small_pool.tile([G, Fc, W], F32, name="r",
                                               tag=f"r{c}")
                        nc.vector.reciprocal(out=r_sb, in_=s_ps)
                        nc.scalar.activation(
                            out=stashes[c][:, n // RS], in_=s_ps,
                            func=mybir.ActivationFunctionType.Identity,
                            scale=1.0)
                        rb_ps = bps_pool.tile([P, Fc, W], F32, name="rbp",
                                              tag="bps")
                        mm512(rb_ps, bcast, r_sb, Fc * W)
                        rb_sb = small_pool.tile([P, Fc, W], BF16, name="rb",
                                                tag=f"rb{c}")
                        nc.scalar.activation(
                            out=rb_sb, in_=rb_ps,
                            func=mybir.ActivationFunctionType.Identity,
                            scale=1.0)
                        nc.gpsimd.tensor_mul(
                            out=new_ec[:, :, 3, :], in0=new_ec[:, :, 3, :],
                            in1=rb_sb)
                if new_ec is not None:
                    ecs[c] = new_ec

        # ---- per-chain finalization via the stash ----
        for c in range(C):
            Fc = CHAINS[c][1]
            fin_ps = bps_pool.tile([G, Fc, W], F32, name="fin", tag="bps")
            mm512(fin_ps, ones_bd, alphas[c], Fc * W)
            nc.scalar.activation(out=stashes[c][:, NBUND + 1], in_=fin_ps,
                                 func=mybir.ActivationFunctionType.Identity,
                                 scale=1.0)
            if S - 1 in SEGMAP[c]:
                # last segment: astop-weighted final sum overrides its slot
                fin2 = bps_pool.tile([G, W], F32, name="fin2", tag="bps")
                nc.tensor.matmul(fin2, astop, alphas[c][:, Fc - 1, :],
                                 start=True, stop=True)
                nc.scalar.activation(
                    out=stashes[c][:, NBUND + 1, Fc - 1, :], in_=fin2,
                    func=mybir.ActivationFunctionType.Identity, scale=1.0)
        # batched Lns (one activation-table load), then Pool combines
        for c in range(C):
            Fc = CHAINS[c][1]
            st = stashes[c]
            lnst = st  # Ln applied in place (ACT reads then writes each elem)
            nc.scalar.activation(out=lnst[:, :, :, 0:J], in_=st[:, :, :, 0:J],
                                 func=mybir.ActivationFunctionType.Ln,
                                 scale=1.0)
            for sl, sc in [((0,), SBU1_B), (tuple(range(1, NBUND)), SBU_B),
                           ((NBUND,), SBI_B), ((NBUND + 1,), SFI_B)]:
                lo, hi = sl[0], sl[-1] + 1
                nc.scalar.activation(
                    out=lnst[:, lo:hi, :, J:W], in_=st[:, lo:hi, :, J:W],
                    func=mybir.ActivationFunctionType.Ln,
                    scale=float(math.exp(sc)))
            acc = small_pool.tile([G, Fc, W], F32, name="acc", tag=f"acc{c}")
            nc.gpsimd.tensor_add(out=acc, in0=lnst[:, 0], in1=lnst[:, 1])
            for i in range(2, NBUND):
                nc.gpsimd.tensor_add(out=acc, in0=acc, in1=lnst[:, i])
            nc.gpsimd.tensor_add(out=acc, in0=acc, in1=lnst[:, NBUND + 1])
            nc.gpsimd.tensor_sub(out=ress[c], in0=acc, in1=lnst[:, NBUND])

        # ---- assembly: nll = sum_s res_alpha - sum_s res_beta + const ----
        tot = small_pool.tile([G, W], F32, name="tot")
        nc.gpsimd.tensor_add(out=tot, in0=ress[0][:, 0, :], in1=ress[0][:, 1, :])
        for c in range(C):
            k0 = 2 if c == 0 else 0
            for k in range(k0, CHAINS[c][1]):
                nc.gpsimd.tensor_add(out=tot, in0=tot, in1=ress[c][:, k, :])
        nll_sb = small_pool.tile([G, J], F32, name="nll_sb")
        nc.gpsimd.tensor_sub(out=nll_sb, in0=tot[:, 0:J], in1=tot[:, J:W])
        nc.gpsimd.tensor_scalar_add(out=nll_sb, in0=nll_sb,
                                    scalar1=float(NLL_CONST))
        nc.sync.dma_start(out=out_ext[:], in_=nll_sb)

    nc.finalize()
    return nc


def _host_prep(feats, tags, transitions):
    """Build per-core input maps. Layout/dtype staging only — all FLOPs on
    device except the 11x11 exp(transitions) weight build."""
    import ml_dtypes
    f32 = np.float32
    bf16 = ml_dtypes.bfloat16
    feats = np.asarray(feats, dtype=f32)
    tags_i = np.asarray(tags).astype(np.int32)
    trans = np.asarray(transitions, dtype=f32)

    def padp(a):
        out = np.zeros((P,) + a.shape[1:], dtype=a.dtype)
        out[:PL] = a
        return np.ascontiguousarray(out)

    A = np.exp(trans.astype(np.float64)).astype(f32)     # A[next, prev]
    Abody = A[:KT, :KT]
    eye = np.eye(G, dtype=f32)
    bd0 = np.kron(eye, Abody.T)
    bd = np.zeros((P, P), dtype=bf16)
    bd[:PL, :PL] = bd0.astype(bf16)
    astart = padp(np.tile(A[:KT, START], G)[:, None].astype(f32))
    astop = padp(np.kron(eye, A[STOP, :KT].reshape(KT, 1)).astype(bf16))
    ones_bd = padp(np.kron(eye, np.ones((KT, 1), f32)).astype(bf16))
    bcast = np.zeros((G, P), dtype=f32)
    bcast[:, :PL] = np.kron(eye, np.ones((1, KT), f32))

    nslots = G * J
    in_maps = []
    for core in range(NCORES):
        fb = feats[core * BL:(core + 1) * BL, :, :KT]
        tb = tags_i[core * BL:(core + 1) * BL]
        fpad = np.zeros((nslots, T, KT), dtype=f32)
        fpad[:BL] = fb
        tpad_s = np.zeros((nslots, T), dtype=np.int32)
        tpad_s[:BL] = tb
        ftime = np.zeros((nslots, TP, KT), dtype=f32)
        ftime[:, BI:BI + T] = fpad
        feats_T = padp(np.ascontiguousarray(
            ftime.reshape(G, J, TP, KT).transpose(0, 3, 2, 1)
            .reshape(PL, TP, J)).astype(bf16))
        ttime = np.zeros((nslots, TP), dtype=np.int32)
        ttime[:, BI:BI + T] = tpad_s
        ttime[:, :BI] = tpad_s[:, :1]
        tg_gj = ttime.reshape(G, J, TP)

        core_map = {
            "feats_t": feats_T,
            "bd_lhst": bd,
            "astart": astart,
            "astop": astop,
            "ones_bd": ones_bd,
            "bcast": bcast,
        }
        for c in range(C):
            Fc = CHAINS[c][1]
            segs = _seg_list(c)
            ohm = np.zeros((P, NB, Fc, LB, J), dtype=np.uint8)
            init_st = np.zeros((P, Fc, W), dtype=bf16)
            init_st[:PL, :, 0:J] = 1.0
            for k, s in enumerate(segs):
                win = tg_gj[:, :, s * SEG:s * SEG + N]    # [G, J, N]
                oh = (np.arange(KT)[:, None, None, None] ==
                      win[None]).astype(np.uint8)          # [KT, G, J, N]
                # [KT, G, J, NB, LB] -> [(G,KT), NB, LB, J]
                ohm[:PL, :, k] = (oh.reshape(KT, G, J, NB, LB)
                                  .transpose(1, 0, 3, 4, 2)
                                  .reshape(PL, NB, LB, J))
                prev = ttime[:, s * SEG - 1] if s > 0 else ttime[:, 0]
                onehot = (np.arange(KT)[:, None, None] ==
                          prev.reshape(1, G, J)).astype(f32)
                init_st[:PL, k, J:W] = (
                    onehot.transpose(1, 0, 2).reshape(PL, J).astype(bf16))
            core_map[f"onehot{c}"] = ohm
            core_map[f"init_st{c}"] = init_st
        in_maps.append(core_map)
    return in_maps


LAST_EXEC_NS = None


def kernel(feats, tags, transitions):
    global LAST_EXEC_NS
    in_maps = _host_prep(feats, tags, transitions)
    nc = _build_nc()
    trace = os.environ.get("KERNEL_TRACE") == "1"
    res = None
    for attempt in range(3):
        try:
            res = run_bass_kernel_spmd(
                nc, in_maps, list(range(NCORES)), trace=trace)
            break
        except Exception:
            if attempt == 2:
                raise
            import time as _time
            import jax as _jax
            try:
                _jax.clear_caches()
            except Exception:
                pass
            for fn in ("clear_backends",):
                try:
                    getattr(_jax.extend.backend, fn)()
                except Exception:
                    try:
                        getattr(_jax, fn)()
                    except Exception:
                        pass
            _time.sleep(5)
    LAST_EXEC_NS = res.exec_time_ns
    outs = []
    for core in range(NCORES):
        nll_parts = np.asarray(res.results[core]["nll"], dtype=np.float32)
        outs.append(nll_parts.reshape(-1)[:BL])
    return np.concatenate(outs).astype(np.float32)


if __name__ == "__main__":
    rng = np.random.default_rng(0)
    feats = rng.standard_normal((B, T, K), dtype=np.float32)
    tags = rng.integers(0, 9, size=(B, T), dtype=np.int64)
    trans = rng.random((K, K), dtype=np.float32)
    trans[START, :] = -10000.0
    trans[:, STOP] = -10000.0
    out = kernel(feats=feats, tags=tags, transitions=trans)
    print(out.shape, out[:4])


# revision 34
# speedup vs baseline: 4.4253x; 1.0047x over previous
"""CRF NLL loss kernel for Trainium2 (8 NeuronCores, batch-parallel).

Strategy (v3): time-segmented forward recursion. The per-step chain
matmul(PE) -> emission-multiply has ~430ns serial latency, so a single
T=2048 chain is latency-bound (~1ms). The CRF transfer operator is strongly
contracting (Birkhoff coefficient <= tanh(0.5) ~ 0.46/step since
log A in [0,1)), so the forward direction forgets its init in ~8 steps. We
split T into S=16 segments of 128 steps; each segment chain starts BI=8
steps early (alpha: uniform init; gold/beta: exact one-hot at the known
gold tag) and per-segment results are stitched by a telescoping boundary
correction validated in proto.py:

  logX = sum_s ln( fin_sum_s * prod(rescale sums after start) / sum@start )

Chains fuse F segments per instruction and run staggered. GPSIMD cannot
touch PSUM on real HW, so the PSUM crossing (matmul output -> SBUF) runs on
DVE ("v" chains: fused multiply) or ACT ("a" chains: copy, then Pool does
the SBUF multiply). The gold half's masked emission ec_beta = onehot *
ec_alpha is built by Pool from host-packed one-hot BITS:
(packed & (1<<q)) * ec_alpha — the stray 2^q factor is a compile-time
constant absorbed into the final nll offset. Scale accumulators are kept in
product form (one Ln per chain at the end) to avoid ACT function-table
swaps; per-half centering immediates keep everything in f32/bf16 range.
"""
import math
import os
import sys

import numpy as np

sys.path.insert(0, "/opt/trn_rl_repo")

from contextlib import ExitStack

import concourse.bacc as bacc
import concourse.bass as bass
import concourse.tile as tile
from concourse import mybir
from concourse.bass_utils import run_bass_kernel_spmd

# problem constants (hardcoded per spec)
B, T, K = 4096, 2048, 11
START, STOP = 10, 9
NCORES = 8
BL = B // NCORES          # 512 sentences per core
G, KT, J = 14, 9, 37      # groups x body-tags x sentences-per-group (518 slots)
P = 128
PL = G * KT               # live partitions
W = 2 * J                 # 74 free per segment: [alpha | beta]

# segmentation
S = 16                    # segments
SEG = T // S              # 128 official steps per segment
BI = 4                    # burn-in steps
N = SEG + BI              # chain steps
LB = 4                    # steps per ec/ft block
NB = N // LB              # blocks per chain
RS = 16                   # rescale cadence (beta ~e^-2.7/step)
RPH = 11                  # rescale at n in {11,27,...,123}
NBUND = 8                 # bundles per chain
TP = 17 * SEG             # padded time length (BI junk + T + tail junk)
C0A = 3.2                 # per-step log recentering (shared by both halves)

# chain layout: list of (cross_engine, segments). "v": DVE fused multiply;
# "a": ACT copy + Pool multiply. Each chain's segments must be equally
# spaced (the block DMA uses one strided access pattern).
SEGMAP = [[0, 2, 4, 6, 8, 10, 12, 14], [1, 3, 5, 7, 9, 11, 13, 15]]
KINDS = ["v", "v"]
C = len(SEGMAP)
CHAINS = [(KINDS[c], len(SEGMAP[c])) for c in range(C)]
assert sorted(s for segs in SEGMAP for s in segs) == list(range(S))
for segs in SEGMAP:
    assert len(set(np.diff(segs))) <= 1

# Ln LUT inputs are pre-scaled to land near 1 (the LUT loses precision at
# extreme magnitudes). The alpha half stays near 1 (scale 1.0); the beta
# (gold one-hot) half decays ~e^-2.7/step: its sums sit near e^-21 at the
# snapshot (~8 steps), e^-32 at the first bundle, e^-42 at later bundles
# (16-step windows) and e^-32 at the final read (12 steps). The ln(scale)
# offsets are per-column-class constants folded into NLL_CONST.
SBI_B, SBU1_B, SBU_B, SFI_B = 13.0, 32.0, 42.0, 32.0
BETA_LN_SUM = SBU1_B + (NBUND - 1) * SBU_B + SFI_B
# every beta segment column: res_b = true + (-SBI_B + BETA_LN_SUM)
# (segment 0's snapshot slot is memset to 1.0, so its -SBI_B is constant too)
NLL_CONST = S * (-SBI_B + BETA_LN_SUM)

F32 = mybir.dt.float32
BF16 = mybir.dt.bfloat16
U8 = mybir.dt.uint8


def _seg_list(c):
    return SEGMAP[c]


def _build_nc(nrep=1):
    nc = bacc.Bacc()
    f_in = nc.declare_dram_parameter("feats_t", [P, TP, J], BF16, isOutput=False)
    oh_ins = [nc.declare_dram_parameter(f"onehot{c}",
                                        [P, NB, CHAINS[c][1], LB, J],
                                        U8, isOutput=False) for c in range(C)]
    bd_in = nc.declare_dram_parameter("bd_lhst", [P, P], BF16, isOutput=False)
    astart_in = nc.declare_dram_parameter("astart", [P, 1], F32, isOutput=False)
    astop_in = nc.declare_dram_parameter("astop", [P, G], BF16, isOutput=False)
    ones_in = nc.declare_dram_parameter("ones_bd", [P, G], BF16, isOutput=False)
    bcast_in = nc.declare_dram_parameter("bcast", [G, P], F32, isOutput=False)
    init_ins = [nc.declare_dram_parameter(f"init_st{c}", [P, CHAINS[c][1], W],
                                          BF16, isOutput=False)
                for c in range(C)]
    out_ext = nc.declare_dram_parameter("nll", [G, J], F32, isOutput=True)
    dbg_steps = [0, 1, 2, 7, 8, 9, 19, 20] if os.environ.get("KDBG") else []
    dbg_outs = {n: nc.declare_dram_parameter(f"dbg{n}", [P, CHAINS[1][1], W],
                                             F32, isOutput=True)
                for n in dbg_steps}
    dbg_ec = {n: nc.declare_dram_parameter(f"dbgec{n}", [P, CHAINS[1][1], W],
                                           F32, isOutput=True)
              for n in dbg_steps}
    dbg_acc = [nc.declare_dram_parameter(f"dbgacc{c}", [G, CHAINS[c][1], W],
                                         F32, isOutput=True)
               for c in range(C)] if dbg_steps else []
    dbg_res = [nc.declare_dram_parameter(f"dbgres{c}", [G, CHAINS[c][1], W],
                                         F32, isOutput=True)
               for c in range(C)] if dbg_steps else []

    f_r = f_in.rearrange("p (w q) j -> p w q j", q=SEG)   # [P, 17, 128, J]

    with tile.TileContext(nc) as tc, ExitStack() as ctx:
        consts = ctx.enter_context(tc.tile_pool(name="consts", bufs=1))
        ft_pools = [ctx.enter_context(tc.tile_pool(name=f"ft{c}", bufs=2))
                    for c in range(C)]
        ec_pools = [ctx.enter_context(tc.tile_pool(name=f"ec{c}", bufs=2))
                    for c in range(C)]
        state_pools = [ctx.enter_context(tc.tile_pool(name=f"st{c}", bufs=3))
                       for c in range(C)]
        sbx_pools = [ctx.enter_context(tc.tile_pool(name=f"sbx{c}", bufs=2))
                     for c in range(C)]
        small_pool = ctx.enter_context(tc.tile_pool(name="small", bufs=2))
        cps_pools = [ctx.enter_context(
            tc.tile_pool(name=f"cpsum{c}", bufs=1, space="PSUM"))
            for c in range(C)]
        bps_pool = ctx.enter_context(
            tc.tile_pool(name="bpsum", bufs=2, space="PSUM"))

        # ---- constants ----
        bd = consts.tile([P, P], BF16, name="bd")
        nc.sync.dma_start(out=bd, in_=bd_in[:])
        astart = consts.tile([P, 1], F32, name="astart")
        nc.sync.dma_start(out=astart, in_=astart_in[:])
        astop = consts.tile([P, G], BF16, name="astop")
        nc.sync.dma_start(out=astop, in_=astop_in[:])
        ones_bd = consts.tile([P, G], BF16, name="ones_bd")
        nc.sync.dma_start(out=ones_bd, in_=ones_in[:])
        bcast = consts.tile([G, P], F32, name="bcast")
        nc.sync.dma_start(out=bcast, in_=bcast_in[:])
        bias_a = consts.tile([P, 1], F32, name="bias_a")
        nc.vector.memset(bias_a, -C0A)
        oh_pools = [ctx.enter_context(tc.tile_pool(name=f"oh{c}", bufs=2))
                    for c in range(C)]
        OHQ = 3                           # blocks per one-hot staging DMA
        oh_tiles = [None] * C

        def stage_onehot(c, b):
            Fc = CHAINS[c][1]
            nblk = min(OHQ, NB - b)
            oh = oh_pools[c].tile([P, OHQ, Fc, LB, J], U8, name="oh",
                                  tag="oh")
            nc.scalar.dma_start(out=oh[:, 0:nblk],
                                in_=oh_ins[c][:, b:b + nblk])
            return oh

        NST = NBUND + 2               # stash slots: bundles, snapshot, fin
        stashes, ress = [], []
        for c in range(C):
            Fc = CHAINS[c][1]
            st = consts.tile([G, NST, Fc, W], F32, name="st", tag=f"st{c}")
            stashes.append(st)
            res = consts.tile([G, Fc, W], F32, name="res", tag=f"res{c}")
            ress.append(res)

        def mm512(out, lhsT, rhs, cols):
            """matmul in <=512-column pieces (PSUM bank limit)."""
            of = out.rearrange("p f w -> p (f w)") if len(out.shape) == 3 else out
            rf = rhs.rearrange("p f w -> p (f w)") if len(rhs.shape) == 3 else rhs
            for lo in range(0, cols, 512):
                hi = min(lo + 512, cols)
                nc.tensor.matmul(of[:, lo:hi], lhsT, rf[:, lo:hi],
                                 start=True, stop=True)

        def chain_mm(ps, al, Fc):
            mm512(ps, bd, al, Fc * W)

        def make_block(c, b):
            """DMA + exp + beta-mask unpack for chain c, block b."""
            Fc = CHAINS[c][1]
            ft = ft_pools[c].tile([P, Fc, LB, J], BF16, name="ft", tag="ft")
            woff, qoff = (b * LB) // SEG, (b * LB) % SEG
            segs = SEGMAP[c]
            w0 = segs[0] + woff
            stride = segs[1] - segs[0] if Fc > 1 else 1
            src = f_r[:, w0:w0 + stride * (Fc - 1) + 1:stride,
                      qoff:qoff + LB, :]
            nc.sync.dma_start(out=ft, in_=src)
            ec = ec_pools[c].tile([P, Fc, LB, W], BF16, name="ec", tag="ec")
            nc.scalar.activation(
                out=ec[:, :, :, 0:J], in_=ft,
                func=mybir.ActivationFunctionType.Exp, bias=bias_a, scale=1.0)
            if b % OHQ == 0:
                oh_tiles[c] = stage_onehot(c, b)
            nc.gpsimd.tensor_mul(out=ec[:, :, :, J:W],
                                 in0=oh_tiles[c][:, b % OHQ],
                                 in1=ec[:, :, :, 0:J])
            return ec

        # ---- initial state + first blocks ----
        alphas, ecs = [], []
        for c in range(C):
            Fc = CHAINS[c][1]
            al = state_pools[c].tile([P, Fc, W], BF16, name="al", tag="al")
            nc.sync.dma_start(out=al, in_=init_ins[c][:])
            alphas.append(al)
            ecs.append(make_block(c, 0))


        for rep in range(nrep):
          for b in range(NB):
            for c in range(C):
                Fc = CHAINS[c][1]
                kind = CHAINS[c][0]
                new_ec = make_block(c, b + 1) if b + 1 < NB else None
                ec = ecs[c]
                for q in range(LB):
                    n = b * LB + q
                    if n == BI:
                        # boundary snapshot: 1/sum of state BEFORE step t_s
                        bi_ps = bps_pool.tile([G, Fc, W], F32, name="bi",
                                              tag="bps")
                        mm512(bi_ps, ones_bd, alphas[c], Fc * W)
                        nc.scalar.activation(
                            out=stashes[c][:, NBUND], in_=bi_ps,
                            func=mybir.ActivationFunctionType.Identity,
                            scale=1.0)
                        if c == 0:
                            nc.vector.memset(stashes[c][:, NBUND, 0, :], 1.0)

                    ps = cps_pools[c].tile([P, Fc, W], F32, name="ps",
                                           tag="ps")
                    chain_mm(ps, alphas[c], Fc)
                    alphas[c] = state_pools[c].tile([P, Fc, W], BF16,
                                                    name="al", tag="al")
                    if kind == "v":
                        nc.vector.tensor_mul(out=alphas[c], in0=ps,
                                             in1=ec[:, :, q, :])
                    else:
                        sb = sbx_pools[c].tile([P, Fc, W], BF16, name="sb",
                                               tag="sb")
                        nc.scalar.activation(
                            out=sb, in_=ps,
                            func=mybir.ActivationFunctionType.Identity,
                            scale=1.0)
                        nc.gpsimd.tensor_mul(out=alphas[c], in0=sb,
                                             in1=ec[:, :, q, :])

                    if n == BI and c == 0:
                        # seg 0: exact re-init (t==0 path), both halves
                        nc.vector.tensor_scalar_mul(
                            out=alphas[c][:, 0, :], in0=ec[:, 0, q, :],
                            scalar1=astart)

                    if c == 1 and n in dbg_outs:
                        dal = small_pool.tile([P, Fc, W], F32, name="dal",
                                              tag="dal")
                        nc.vector.tensor_scalar_mul(out=dal, in0=alphas[c],
                                                    scalar1=1.0)
                        nc.sync.dma_start(out=dbg_outs[n][:], in_=dal)
                        dec = small_pool.tile([P, Fc, W], F32, name="dec",
                                              tag="dec")
                        nc.vector.tensor_scalar_mul(out=dec,
                                                    in0=ec[:, :, q, :],
                                                    scalar1=1.0)
                        nc.sync.dma_start(out=dbg_ec[n][:], in_=dec)

                    if n % RS == RPH:
                        s_ps = bps_pool.tile([G, Fc, W], F32, name="sps",
                                             tag="bps")
                        mm512(s_ps, ones_bd, alphas[c], Fc * W)
                        r_sb = small_pool.tile([G, Fc, W], F32, name="r",
                                               tag=f"r{c}")
                        nc.vector.reciprocal(out=r_sb, in_=s_ps)
                        nc.scalar.activation(
                            out=stashes[c][:, n // RS], in_=s_ps,
                            func=mybir.ActivationFunctionType.Identity,
                            scale=1.0)
                        rb_ps = bps_pool.tile([P, Fc, W], F32, name="rbp",
                                              tag="bps")
                        mm512(rb_ps, bcast, r_sb, Fc * W)
                        rb_sb = small_pool.tile([P, Fc, W], BF16, name="rb",
                                                tag=f"rb{c}")
                        nc.scalar.activation(
                            out=rb_sb, in_=rb_ps,
                            func=mybir.ActivationFunctionType.Identity,
                            scale=1.0)
                        nc.gpsimd.tensor_mul(
                            out=new_ec[:, :, 3, :], in0=new_ec[:, :, 3, :],
                            in1=rb_sb)
                if new_ec is not None:
                    ecs[c] = new_ec

        # ---- per-chain finalization via the stash ----
        for c in range(C):
            Fc = CHAINS[c][1]
            fin_ps = bps_pool.tile([G, Fc, W], F32, name="fin", tag="bps")
            mm512(fin_ps, ones_bd, alphas[c], Fc * W)
            nc.scalar.activation(out=stashes[c][:, NBUND + 1], in_=fin_ps,
                                 func=mybir.ActivationFunctionType.Identity,
                                 scale=1.0)
            if S - 1 in SEGMAP[c]:
                # last segment: astop-weighted final sum overrides its slot
                fin2 = bps_pool.tile([G, W], F32, name="fin2", tag="bps")
                nc.tensor.matmul(fin2, astop, alphas[c][:, Fc - 1, :],
                                 start=True, stop=True)
                nc.scalar.activation(
                    out=stashes[c][:, NBUND + 1, Fc - 1, :], in_=fin2,
                    func=mybir.ActivationFunctionType.Identity, scale=1.0)
        # batched Lns (one activation-table load), then Pool combines
        for c in range(C):
            Fc = CHAINS[c][1]
            st = stashes[c]
            lnst = st  # Ln applied in place (ACT reads then writes each elem)
            nc.scalar.activation(out=lnst[:, :, :, 0:J], in_=st[:, :, :, 0:J],
                                 func=mybir.ActivationFunctionType.Ln,
                                 scale=1.0)
            for sl, sc in [((0,), SBU1_B), (tuple(range(1, NBUND)), SBU_B),
                           ((NBUND,), SBI_B), ((NBUND + 1,), SFI_B)]:
                lo, hi = sl[0], sl[-1] + 1
                nc.scalar.activation(
                    out=lnst[:, lo:hi, :, J:W], in_=st[:, lo:hi, :, J:W],
                    func=mybir.ActivationFunctionType.Ln,
                    scale=float(math.exp(sc)))
            acc = small_pool.tile([G, Fc, W], F32, name="acc", tag=f"acc{c}")
            nc.gpsimd.tensor_add(out=acc, in0=lnst[:, 0], in1=lnst[:, 1])
            for i in range(2, NBUND):
                nc.gpsimd.tensor_add(out=acc, in0=acc, in1=lnst[:, i])
            nc.gpsimd.tensor_add(out=acc, in0=acc, in1=lnst[:, NBUND + 1])
            nc.gpsimd.tensor_sub(out=ress[c], in0=acc, in1=lnst[:, NBUND])

        # ---- assembly: nll = sum_s res_alpha - sum_s res_beta + const ----
        tot = small_pool.tile([G, W], F32, name="tot")
        nc.gpsimd.tensor_add(out=tot, in0=ress[0][:, 0, :], in1=ress[0][:, 1, :])
        for c in range(C):
            k0 = 2 if c == 0 else 0
            for k in range(k0, CHAINS[c][1]):
                nc.gpsimd.tensor_add(out=tot, in0=tot, in1=ress[c][:, k, :])
        nll_sb = small_pool.tile([G, J], F32, name="nll_sb")
        nc.gpsimd.tensor_sub(out=nll_sb, in0=tot[:, 0:J], in1=tot[:, J:W])
        nc.gpsimd.tensor_scalar_add(out=nll_sb, in0=nll_sb,
                                    scalar1=float(NLL_CONST))
        nc.sync.dma_start(out=out_ext[:], in_=nll_sb)

    nc.finalize()
    return nc


def _host_prep(feats, tags, transitions):
    """Build per-core input maps. Layout/dtype staging only — all FLOPs on
    device except the 11x11 exp(transitions) weight build."""
    import ml_dtypes
    f32 = np.float32
    bf16 = ml_dtypes.bfloat16
    feats = np.asarray(feats, dtype=f32)
    tags_i = np.asarray(tags).astype(np.int32)
    trans = np.asarray(transitions, dtype=f32)

    def padp(a):
        out = np.zeros((P,) + a.shape[1:], dtype=a.dtype)
        out[:PL] = a
        return np.ascontiguousarray(out)

    A = np.exp(trans.astype(np.float64)).astype(f32)     # A[next, prev]
    Abody = A[:KT, :KT]
    eye = np.eye(G, dtype=f32)
    bd0 = np.kron(eye, Abody.T)
    bd = np.zeros((P, P), dtype=bf16)
    bd[:PL, :PL] = bd0.astype(bf16)
    astart = padp(np.tile(A[:KT, START], G)[:, None].astype(f32))
    astop = padp(np.kron(eye, A[STOP, :KT].reshape(KT, 1)).astype(bf16))
    ones_bd = padp(np.kron(eye, np.ones((KT, 1), f32)).astype(bf16))
    bcast = np.zeros((G, P), dtype=f32)
    bcast[:, :PL] = np.kron(eye, np.ones((1, KT), f32))

    nslots = G * J
    in_maps = []
    for core in range(NCORES):
        fb = feats[core * BL:(core + 1) * BL, :, :KT]
        tb = tags_i[core * BL:(core + 1) * BL]
        fpad = np.zeros((nslots, T, KT), dtype=f32)
        fpad[:BL] = fb
        tpad_s = np.zeros((nslots, T), dtype=np.int32)
        tpad_s[:BL] = tb
        ftime = np.zeros((nslots, TP, KT), dtype=f32)
        ftime[:, BI:BI + T] = fpad
        feats_T = padp(np.ascontiguousarray(
            ftime.reshape(G, J, TP, KT).transpose(0, 3, 2, 1)
            .reshape(PL, TP, J)).astype(bf16))
        ttime = np.zeros((nslots, TP), dtype=np.int32)
        ttime[:, BI:BI + T] = tpad_s
        ttime[:, :BI] = tpad_s[:, :1]
        tg_gj = ttime.reshape(G, J, TP)

        core_map = {
            "feats_t": feats_T,
            "bd_lhst": bd,
            "astart": astart,
            "astop": astop,
            "ones_bd": ones_bd,
            "bcast": bcast,
        }
        for c in range(C):
            Fc = CHAINS[c][1]
            segs = _seg_list(c)
            ohm = np.zeros((P, NB, Fc, LB, J), dtype=np.uint8)
            init_st = np.zeros((P, Fc, W), dtype=bf16)
            init_st[:PL, :, 0:J] = 1.0
            for k, s in enumerate(segs):
                win = tg_gj[:, :, s * SEG:s * SEG + N]    # [G, J, N]
                oh = (np.arange(KT)[:, None, None, None] ==
                      win[None]).astype(np.uint8)          # [KT, G, J, N]
                # [KT, G, J, NB, LB] -> [(G,KT), NB, LB, J]
                ohm[:PL, :, k] = (oh.reshape(KT, G, J, NB, LB)
                                  .transpose(1, 0, 3, 4, 2)
                                  .reshape(PL, NB, LB, J))
                prev = ttime[:, s * SEG - 1] if s > 0 else ttime[:, 0]
                onehot = (np.arange(KT)[:, None, None] ==
                          prev.reshape(1, G, J)).astype(f32)
                init_st[:PL, k, J:W] = (
                    onehot.transpose(1, 0, 2).reshape(PL, J).astype(bf16))
            core_map[f"onehot{c}"] = ohm
            core_map[f"init_st{c}"] = init_st
        in_maps.append(core_map)
    return in_maps


LAST_EXEC_NS = None


def kernel(feats, tags, transitions):
    global LAST_EXEC_NS
    in_maps = _host_prep(feats, tags, transitions)
    nc = _build_nc()
    trace = os.environ.get("KERNEL_TRACE") == "1"
    res = None
    for attempt in range(3):
        try:
            res = run_bass_kernel_spmd(
                nc, in_maps, list(range(NCORES)), trace=trace)
            break
        except Exception:
            if attempt == 2:
                raise
            import time as _time
            import jax as _jax
            try:
                _jax.clear_caches()
            except Exception:
                pass
            for fn in ("clear_backends",):
                try:
                    getattr(_jax.extend.backend, fn)()
                except Exception:
                    try:
                        getattr(_jax, fn)()
                    except Exception:
                        pass
            _time.sleep(5)
    LAST_EXEC_NS = res.exec_time_ns
    outs = []
    for core in range(NCORES):
        nll_parts = np.asarray(res.results[core]["nll"], dtype=np.float32)
        outs.append(nll_parts.reshape(-1)[:BL])
    return np.concatenate(outs).astype(np.float32)


if __name__ == "__main__":
    rng = np.random.default_rng(0)
    feats = rng.standard_normal((B, T, K), dtype=np.float32)
    tags = rng.integers(0, 9, size=(B, T), dtype=np.int64)
    trans = rng.random((K, K), dtype=np.float32)
    trans[START, :] = -10000.0
    trans[:, STOP] = -10000.0
    out = kernel(feats=feats, tags=tags, transitions=trans)
    print(out.shape, out[:4])
